# revision 47
# baseline (speedup 1.0000x reference)
"""DKVMN forward kernel for Trainium2 (8 NeuronCores, batch-parallel).

Per core (8 batches, b = 4*h2 + 2*j + c):
  Phase A: per-tile indirect-DMA gathers, PE transposes -> kT/vT,
    softmax attention w (Exp table load once), gates e/a (Sigmoid/Tanh
    table once).  Compact bf16 images of w and [e|a] are stored to
    DRAM row-major (plain [128, N] copies, ~500ns each).
  Scan (t = 0..199, chunks of CH=20): per chunk 12 contiguous refill
    DMAs (spread over SP/Pool queues) load w9_j [4, CH*128] /
    ea9_j [4, CH*512] bf16 tiles (pad halves zeroed once at startup).
    Per step: 2 bf16 matmuls produce PSUM [A|B] = [w*e | w*a] blocks
    (213ns each), 1 fp32r matmul produces reads psr [8, 512]; DVE does
    T = (A-1)*Mv ; Mv' = B - T (= (1-we)Mv + wa, no ones-row needed);
    one ACT copy drains psr to a bf16 ring, shipped to a DRAM reads
    image every 10 steps (ACT queue).  GPSIMD pulls finished row-tiles
    back as [32,128] partition-aligned loads during the scan.  Steady
    state is DVE-bound at 2x658ns/step; all other engines have slack.
  Phase C: 13 PE transposes + permuted DVE copies build readsT
    [k, (t,b)], fp32r matmuls with f_W halves, Tanh (+bias), p_W
    matvec, Sigmoid, one output DMA of [1, 1600].

  Hardware notes (sim-correct variants that FAIL on real TRN2):
  - multi-column indirect-gather offsets scramble; use [128,1] per tile
  - compute-engine SBUF operands need start partition in {0,32,64,96}
  - fp32r matmul inputs must come from rounding producers (not raw DMA)
  - GPSIMD cannot touch PSUM; f32r memset unsupported
  - partition-dim-split SBUF APs on DMAs lose dependency tracking
"""

import numpy as np

B, S, DK, DV, NQ = 64, 200, 128, 64, 10000
NCORES = 8
BL = B // NCORES          # 8 batches per core
TP = 208                  # t padded to 13*16
NTILE = 13                # row tiles of 128 -> 1664 rows
ROWS = TP * BL            # 1664
NOUT = S * BL             # 1600
CH = 20                   # scan chunk length (steps)
NCH = S // CH
RH = 10                   # reads-ring half length (steps)

_CACHE = {}


def _build():
    import concourse.bacc as bacc
    import concourse.bass as bass
    import concourse.mybir as mybir
    from concourse.tile import TileContext
    from concourse.masks import make_identity

    fp32 = mybir.dt.float32
    f32r = mybir.dt.float32r
    bf16 = mybir.dt.bfloat16
    i32 = mybir.dt.int32
    AL = mybir.AluOpType
    AF = mybir.ActivationFunctionType
    AX = mybir.AxisListType

    nc = bacc.Bacc(None)

    QID = nc.dram_tensor("qid32", [TP, 2 * BL], i32, kind="ExternalInput")
    COR = nc.dram_tensor("cor32", [TP, 2 * BL], i32, kind="ExternalInput")
    KEMB = nc.dram_tensor("key_emb", [NQ, DK], fp32, kind="ExternalInput")
    VEMB = nc.dram_tensor("value_emb", [2 * NQ, DK], fp32, kind="ExternalInput")
    MK = nc.dram_tensor("Mk", [DV, DK], fp32, kind="ExternalInput")
    MV0 = nc.dram_tensor("Mv0", [DV, DK], fp32, kind="ExternalInput")
    FW = nc.dram_tensor("f_W", [2 * DK, DK], fp32, kind="ExternalInput")
    FB = nc.dram_tensor("f_b", [DK], fp32, kind="ExternalInput")
    EW = nc.dram_tensor("e_W", [DK, DK], fp32, kind="ExternalInput")
    EB = nc.dram_tensor("e_b", [DK], fp32, kind="ExternalInput")
    AW = nc.dram_tensor("a_W", [DK, DK], fp32, kind="ExternalInput")
    AB_ = nc.dram_tensor("a_b", [DK], fp32, kind="ExternalInput")
    PW = nc.dram_tensor("p_W", [DK, 1], fp32, kind="ExternalInput")
    PB = nc.dram_tensor("p_b", [1], fp32, kind="ExternalInput")

    # compact bf16 scan-operand images, row-major (8t+b, k); b = 4*h2 + 2*j + c
    WIMG = nc.dram_tensor("w_img", [TP * BL, DV], bf16, kind="Internal")
    EAIMG = nc.dram_tensor("ea_img", [TP * BL, 2 * DK], bf16, kind="Internal")
    # reads image: row = psr row (4*h2+2*j+c), col = (t, g, k)
    RIMG = nc.dram_tensor("r_img", [BL, S * 512], bf16, kind="Internal")
    OUT = nc.dram_tensor("out", [1, NOUT], fp32, kind="ExternalOutput")

    with TileContext(nc) as tc:
        with tc.tile_pool(name="const", bufs=1) as const, \
             tc.tile_pool(name="big", bufs=1) as bigp, \
             tc.tile_pool(name="work", bufs=3) as work, \
             tc.tile_pool(name="small", bufs=4) as small, \
             tc.tile_pool(name="mv", bufs=2) as mvp, \
             tc.tile_pool(name="tt", bufs=2) as ttp, \
             tc.tile_pool(name="stor", bufs=NTILE) as storp:

            psA_cm = tc.tile_pool(name="psA", bufs=2, space="PSUM")
            psA = psA_cm.__enter__()
            psW_cm = tc.tile_pool(name="psW", bufs=2, space="PSUM")
            psW = psW_cm.__enter__()
            gath_cm = tc.tile_pool(name="gath", bufs=1)
            gathp = gath_cm.__enter__()

            # ------- scan tiles + one-time memsets (overlap phase A) -------
            w9 = [[bigp.tile([4, CH * 128], bf16, tag=f"w9_{j}_{i}",
                             name=f"w9_{j}_{i}")
                   for i in range(2)] for j in range(2)]
            ea9 = [[bigp.tile([4, CH * 512], bf16, tag=f"ea9_{j}_{i}",
                              name=f"ea9_{j}_{i}")
                    for i in range(2)] for j in range(2)]
            r8 = [bigp.tile([BL, RH * 512], bf16, tag=f"r8_{i}", name=f"r8_{i}")
                  for i in range(2)]
            rrp = [bigp.tile([128, 128], bf16, tag=f"rrp_{i}", name=f"rrp_{i}")
                   for i in range(NTILE)]
            wsel = bigp.tile([128, ROWS], f32r, tag="wsel")

            # ---------------- constants ----------------
            ident = const.tile([128, 128], fp32, tag="ident")
            make_identity(nc, ident[:])
            ident16 = const.tile([128, 128], bf16, tag="ident16")
            make_identity(nc, ident16[:])

            mk_sb = const.tile([DV, DK], fp32, tag="mk_sb")
            nc.sync.dma_start(mk_sb[:], MK[:])
            mkT_ps = psA.tile([128, 512], fp32, space="PSUM", tag="lps")
            nc.tensor.transpose(mkT_ps[0:DK, 0:DV], mk_sb[:], ident[0:DV, 0:DV])
            mkT = const.tile([DK, DV], f32r, tag="mkT")
            nc.vector.tensor_copy(mkT[:], mkT_ps[0:DK, 0:DV])

            ldw = const.tile([DK, 7 * DK + 1], fp32, tag="ldw")
            nc.sync.dma_start(ldw[:, 0:DK], EW[:])
            nc.sync.dma_start(ldw[:, DK:2 * DK], AW[:])
            nc.sync.dma_start(ldw[:, 2 * DK:3 * DK], FW[0:DK, :])
            nc.sync.dma_start(ldw[:, 3 * DK:4 * DK], FW[DK:2 * DK, :])
            nc.sync.dma_start(ldw[:, 4 * DK:4 * DK + 1], PW[:])
            eaW = const.tile([DK, 2 * DK], f32r, tag="eaW")
            nc.vector.tensor_copy(eaW[:], ldw[:, 0:2 * DK])
            fW1 = const.tile([DK, DK], f32r, tag="fW1")
            nc.vector.tensor_copy(fW1[:], ldw[:, 2 * DK:3 * DK])
            fW2 = const.tile([DK, DK], f32r, tag="fW2")
            nc.vector.tensor_copy(fW2[:], ldw[:, 3 * DK:4 * DK])
            pW = const.tile([DK, 1], f32r, tag="pW")
            nc.vector.tensor_copy(pW[:], ldw[:, 4 * DK:4 * DK + 1])
            nc.sync.dma_start(ldw[0:1, 4 * DK + 1:5 * DK + 1],
                              EB[:].rearrange("(o k) -> o k", o=1))
            nc.sync.dma_start(ldw[0:1, 5 * DK + 1:6 * DK + 1],
                              AB_[:].rearrange("(o k) -> o k", o=1))
            eab_row = const.tile([1, 2 * DK], f32r, tag="eab_row")
            nc.vector.tensor_copy(eab_row[:], ldw[0:1, 4 * DK + 1:6 * DK + 1])
            ones_row = const.tile([1, DK], f32r, tag="ones_row")
            nc.vector.memset(ldw[0:1, 6 * DK + 1:7 * DK + 1], 1.0)
            nc.vector.tensor_copy(ones_row[:], ldw[0:1, 6 * DK + 1:7 * DK + 1])
            fb_col = const.tile([DK, 1], fp32, tag="fb_col")
            nc.sync.dma_start(fb_col[:], FB[:].rearrange("(k o) -> k o", o=1))
            pb_t = const.tile([1, 1], fp32, tag="pb_t")
            nc.sync.dma_start(pb_t[:], PB[:].rearrange("(o k) -> o k", o=1))

            mv0_t = const.tile([128, DK], fp32, tag="mv0_t")
            nc.sync.dma_start(mv0_t[0:DV, :], MV0[:])
            nc.sync.dma_start(mv0_t[DV:128, :], MV0[:])

            # ---------------- indices ----------------
            qidx = const.tile([128, 16], i32, tag="qidx")
            cidx = const.tile([128, 16], i32, tag="cidx")
            vidx = const.tile([128, 16], i32, tag="vidx")
            # row r = 128c + p ; p = 8tl + b ; t = 16c + tl
            qsrc = QID[:].rearrange("(c tl) (b two) -> tl b c two", tl=16, two=2)[:, :, :, 0]
            nc.sync.dma_start(qidx[:, 0:NTILE], qsrc)
            csrc = COR[:].rearrange("(c tl) (b two) -> tl b c two", tl=16, two=2)[:, :, :, 0]
            nc.sync.dma_start(cidx[:, 0:NTILE], csrc)
            nc.vector.scalar_tensor_tensor(
                out=vidx[:, 0:NTILE], in0=cidx[:, 0:NTILE], scalar=NQ,
                in1=qidx[:, 0:NTILE], op0=AL.mult, op1=AL.add)

            # ---------------- batched gathers ----------------
            gk = gathp.tile([128, ROWS], fp32, tag="gk")
            # zero wsel via gk as fp32 staging (f32r memset unsupported);
            # the gathers then overwrite gk (write-after-read tracked)
            nc.vector.memset(gk[:], 0.0)
            nc.vector.tensor_copy(wsel[:], gk[:])
            # w9 zeros are small: DVE, early
            for j in range(2):
                nc.vector.memset(w9[j][0][:], 0.0)
                nc.vector.memset(w9[j][1][:], 0.0)
            nc.vector.memset(rrp[NTILE - 1][:], 0.0)
            gv = gathp.tile([128, ROWS], fp32, tag="gv")
            for c in range(NTILE):
                sl = slice(128 * c, 128 * c + 128)
                nc.gpsimd.indirect_dma_start(
                    out=gk[:, sl], out_offset=None, in_=KEMB[:],
                    in_offset=bass.IndirectOffsetOnAxis(ap=qidx[:, c:c + 1], axis=0))
                nc.gpsimd.indirect_dma_start(
                    out=gv[:, sl], out_offset=None, in_=VEMB[:],
                    in_offset=bass.IndirectOffsetOnAxis(ap=vidx[:, c:c + 1], axis=0))

            kT = bigp.tile([DK, ROWS], f32r, tag="kT")
            vT = bigp.tile([DK, ROWS], f32r, tag="vT")

            # ea9 zeros on Pool after the gathers (buffer 0 first: it
            # gates refill(0); buffer 1 is only needed one chunk later)
            for i in range(2):
                for j in range(2):
                    nc.gpsimd.memset(ea9[j][i][:], 0.0)

            # ---------------- loop 1: transposes, softmax w ----------------
            for c in range(NTILE):
                sl = slice(128 * c, 128 * c + 128)
                kt_ps = psA.tile([128, 512], fp32, space="PSUM", tag="kt",
                                 bufs=1)
                nc.tensor.transpose(kt_ps[:, 0:128], gk[:, sl], ident[:])
                nc.vector.tensor_copy(kT[:, sl], kt_ps[:, 0:128])
                vt_ps = psA.tile([128, 512], fp32, space="PSUM", tag="vt",
                                 bufs=1)
                nc.tensor.transpose(vt_ps[:, 0:128], gv[:, sl], ident[:])
                nc.vector.tensor_copy(vT[:, sl], vt_ps[:, 0:128])

                lps = psA.tile([128, 512], fp32, space="PSUM", tag="lps")
                nc.tensor.matmul(lps[:, 0:DV], lhsT=kT[:, sl], rhs=mkT[:],
                                 start=True, stop=True)
                negmax = small.tile([128, 1], fp32, tag="nm")
                nc.vector.tensor_reduce(negmax[:], lps[:, 0:DV], AX.X, AL.max,
                                        negate=True)
                exp_sb = work.tile([128, DV], fp32, tag="exp")
                sumexp = small.tile([128, 1], fp32, tag="se")
                nc.scalar.activation(exp_sb[:], lps[:, 0:DV], AF.Exp,
                                     bias=negmax[:, 0:1], accum_out=sumexp[:, 0:1])
                rec = small.tile([128, 1], fp32, tag="rec")
                nc.vector.reciprocal(rec[:], sumexp[:])
                w16 = storp.tile([128, DV], bf16, tag="w16")
                nc.vector.tensor_scalar_mul(w16[:], exp_sb[:], rec[:, 0:1])

                # w image store: plain row-major copy (GPSIMD queue)
                nc.gpsimd.dma_start(WIMG[128 * c:128 * c + 128, :], w16[:])

                # wsel: transpose w16 -> [64, 128], scatter halves by c-parity
                wt_ps = psW.tile([64, 128], bf16, space="PSUM", tag="wps")
                nc.tensor.transpose(wt_ps[:], w16[:], ident16[:])
                wv = wt_ps[:].rearrange("p (t b) -> p t b", b=BL)
                wz = wsel[:, sl].rearrange("p (t b) -> p t b", b=BL)
                nc.vector.tensor_copy(wz[0:DV, :, 0::2], wv[:, :, 0::2])
                nc.vector.tensor_copy(wz[DV:128, :, 1::2], wv[:, :, 1::2])

            gath_cm.__exit__(None, None, None)

            # ---------------- loop 2: gates e|a ----------------
            for c in range(NTILE):
                sl = slice(128 * c, 128 * c + 128)
                eps = psA.tile([128, 512], fp32, space="PSUM", tag="eps")
                nc.tensor.matmul(eps[:, 0:2 * DK], lhsT=vT[:, sl], rhs=eaW[:],
                                 start=True, stop=False)
                nc.tensor.matmul(eps[:, 0:2 * DK], lhsT=ones_row[:], rhs=eab_row[:],
                                 start=False, stop=True)
                ea_sb = storp.tile([128, 2 * DK], bf16, tag="easb")
                nc.scalar.activation(ea_sb[:, 0:DK], eps[:, 0:DK], AF.Sigmoid)
                nc.scalar.activation(ea_sb[:, DK:2 * DK], eps[:, DK:2 * DK], AF.Tanh)
                # ea image store: plain row-major copy (SP queue)
                nc.sync.dma_start(EAIMG[128 * c:128 * c + 128, :], ea_sb[:])

            psW_cm.__exit__(None, None, None)
            psA_cm.__exit__(None, None, None)

            # ---------------- scan init ----------------
            mv_cur = mvp.tile([128, 4 * DK], f32r, tag="mv")
            for g in range(4):
                nc.vector.tensor_copy(mv_cur[:, DK * g:DK * g + DK], mv0_t[:])

            psS_cm = tc.tile_pool(name="psS", bufs=2, space="PSUM")
            psS = psS_cm.__enter__()
            psR_cm = tc.tile_pool(name="psR", bufs=2, space="PSUM")
            psR = psR_cm.__enter__()

            wimg_v = WIMG[:].rearrange("(t h bl) k -> h bl t k", h=2, bl=4)
            eaimg_v = EAIMG[:].rearrange("(t h bl) k -> h bl t k", h=2, bl=4)
            rimg_v = RIMG[:].rearrange("r (t g k) -> r t g k", g=4, k=128)

            def refill(ch, spread=False, parts="wea"):
                buf = ch % 2
                t0 = ch * CH
                for j in range(2):
                    for h2 in range(2):
                        if "w" in parts:
                            for c2 in range(2):
                                r = 2 * h2 + c2
                                wdst = w9[j][buf][r:r + 1, :].rearrange(
                                    "p (t x) -> p t x", x=128)[
                                    :, :, 64 * c2:64 * c2 + 64]
                                weng = (nc.sync if j == 0 else nc.scalar) \
                                    if spread else nc.gpsimd
                                weng.dma_start(
                                    wdst, wimg_v[h2, 2 * j + c2, t0:t0 + CH, :])
                        if "ea" in parts:
                            eadst = ea9[j][buf][2 * h2:2 * h2 + 2, :].rearrange(
                                "p (t x) -> p t x", x=512)[
                                :, :, 256 * h2:256 * h2 + 256]
                            eng = nc.scalar if (spread and h2 == 1) else nc.sync
                            eng.dma_start(
                                eadst, eaimg_v[h2, 2 * j:2 * j + 2, t0:t0 + CH, :])

            # ---------------- the scan ----------------
            refill(0, spread=True)
            loaded = 0
            for ch in range(NCH):
                buf = ch % 2
                t0 = ch * CH
                for tl in range(CH):
                    t = t0 + tl
                    psab = psS.tile([128, 1024], fp32, space="PSUM", tag="psab")
                    for j in range(2):
                        nc.tensor.matmul(
                            psab[:, 512 * j:512 * j + 512],
                            lhsT=w9[j][buf][0:4, 128 * tl:128 * tl + 128],
                            rhs=ea9[j][buf][0:4, 512 * tl:512 * tl + 512],
                            start=True, stop=True)
                    psr = psR.tile([8, 512], fp32, space="PSUM", tag="psr")
                    nc.tensor.matmul(psr[:], lhsT=wsel[:, 8 * t:8 * t + 8],
                                     rhs=mv_cur[:], start=True, stop=True)

                    psab_v = psab[:].rearrange("p (g x) -> p g x", g=4)
                    tT = ttp.tile([128, 4 * DK], fp32, tag="tt")
                    # T = (A - 1) * Mv
                    nc.vector.scalar_tensor_tensor(
                        out=tT[:].rearrange("p (g x) -> p g x", g=4),
                        in0=psab_v[:, :, 0:DK], scalar=1.0,
                        in1=mv_cur[:].rearrange("p (g x) -> p g x", g=4),
                        op0=AL.subtract, op1=AL.mult)
                    mv_next = mvp.tile([128, 4 * DK], f32r, tag="mv")
                    # Mv' = B - T = (1 - we) Mv + wa
                    nc.vector.tensor_tensor(
                        out=mv_next[:].rearrange("p (g x) -> p g x", g=4),
                        in0=psab_v[:, :, DK:2 * DK],
                        in1=tT[:].rearrange("p (g x) -> p g x", g=4),
                        op=AL.subtract)
                    mv_cur = mv_next

                    # reads drain: one GPSIMD copy into the ring
                    hb = (t // RH) % 2
                    rtl = t % RH
                    nc.scalar.copy(
                        r8[hb][:, 512 * rtl:512 * rtl + 512], psr[:])
                    if rtl == RH - 1:
                        # ship ring half to DRAM (ACT queue; SP carries ea)
                        th = t - RH + 1
                        nc.scalar.dma_start(
                            RIMG[:, 512 * th:512 * (th + RH)], r8[hb][:])

                if ch + 1 < NCH:
                    refill(ch + 1)

                # pull finished row-tiles back (GPSIMD queue)
                cdone = (CH * (ch + 1) - 16) // 16 if ch + 1 < NCH else NTILE - 1
                for ct in range(loaded, cdone + 1):
                    nt = min(16, S - 16 * ct)
                    for h2 in range(2):
                        for j in range(2):
                            rs = 4 * h2 + 2 * j
                            g = 2 * j + h2
                            q = 2 * h2 + j
                            nc.gpsimd.dma_start(
                                rrp[ct][32 * q:32 * q + 2 * nt, :],
                                rimg_v[rs:rs + 2, 16 * ct:16 * ct + nt, g, :])
                loaded = max(loaded, cdone + 1)

            psR_cm.__exit__(None, None, None)
            psS_cm.__exit__(None, None, None)

            # ---------------- phase C ----------------
            psC_cm = tc.tile_pool(name="psC", bufs=3, space="PSUM")
            psC = psC_cm.__enter__()
            psP_cm = tc.tile_pool(name="psP", bufs=2, space="PSUM")
            psP = psP_cm.__enter__()

            readsT = bigp.tile([DK, NOUT], f32r, tag="readsT")
            for ct in range(NTILE):
                nt = min(16, S - 16 * ct)
                rt_ps = psC.tile([128, 128], bf16, space="PSUM", tag="tps16")
                nc.tensor.transpose(rt_ps[:], rrp[ct][:], ident16[:])
                # psum cols (q, c, t) -> readsT cols (t, b=2q+c)
                src = rt_ps[:].rearrange("p (q x) -> p q x", q=4)[
                    :, :, 0:2 * nt].rearrange("p q (c t) -> p t q c", c=2)
                dst = readsT[:, 128 * ct:128 * ct + 8 * nt].rearrange(
                    "p (t q c) -> p t q c", q=4, c=2)
                nc.vector.tensor_copy(dst, src)

            fT = bigp.tile([DK, NOUT], f32r, tag="fT")
            out_sb = const.tile([1, NOUT], fp32, tag="out_sb")
            for c0 in range(0, NOUT, 512):
                w_ = min(512, NOUT - c0)
                sl = slice(c0, c0 + w_)
                fps = psC.tile([128, 512], fp32, space="PSUM", tag="cps")
                nc.tensor.matmul(fps[:, 0:w_], lhsT=fW1[:], rhs=readsT[:, sl],
                                 start=True, stop=False)
                nc.tensor.matmul(fps[:, 0:w_], lhsT=fW2[:], rhs=kT[:, sl],
                                 start=False, stop=True)
                nc.scalar.activation(fT[:, sl], fps[:, 0:w_], AF.Tanh,
                                     bias=fb_col[:, 0:1])
                pps = psP.tile([1, 512], fp32, space="PSUM", tag="cpr")
                nc.tensor.matmul(pps[0:1, 0:w_], lhsT=pW[:], rhs=fT[:, sl],
                                 start=True, stop=True)
                nc.scalar.activation(out_sb[0:1, sl], pps[0:1, 0:w_], AF.Sigmoid,
                                     bias=pb_t[0:1, 0:1])
            nc.sync.dma_start(OUT[:], out_sb[:])
            psP_cm.__exit__(None, None, None)
            psC_cm.__exit__(None, None, None)

    nc.finalize()
    return nc


def make_in_maps(inputs):
    def prep_idx(a):
        # [BL, S] int -> t-major padded little-endian int32 view [TP, 2*BL]
        a = np.ascontiguousarray(np.asarray(a).astype(np.int64, copy=False).T)  # [S, BL]
        v = a.view(np.int32).reshape(S, 2 * BL)
        out = np.zeros((TP, 2 * BL), np.int32)
        out[:S] = v
        return out

    common = {
        "key_emb": np.ascontiguousarray(inputs["key_emb"], np.float32),
        "value_emb": np.ascontiguousarray(inputs["value_emb"], np.float32),
        "Mk": np.ascontiguousarray(inputs["Mk"], np.float32),
        "Mv0": np.ascontiguousarray(inputs["Mv0"], np.float32),
        "f_W": np.ascontiguousarray(inputs["f_W"], np.float32),
        "f_b": np.ascontiguousarray(inputs["f_b"], np.float32),
        "e_W": np.ascontiguousarray(inputs["e_W"], np.float32),
        "e_b": np.ascontiguousarray(inputs["e_b"], np.float32),
        "a_W": np.ascontiguousarray(inputs["a_W"], np.float32),
        "a_b": np.ascontiguousarray(inputs["a_b"], np.float32),
        "p_W": np.ascontiguousarray(inputs["p_W"], np.float32),
        "p_b": np.ascontiguousarray(inputs["p_b"], np.float32),
    }
    in_maps = []
    for core in range(NCORES):
        bs = slice(core * BL, core * BL + BL)
        m = dict(common)
        m["qid32"] = prep_idx(np.asarray(inputs["question_seq"])[bs])
        m["cor32"] = prep_idx(np.asarray(inputs["correctness_seq"])[bs])
        in_maps.append(m)
    return in_maps


def kernel(**inputs):
    from concourse.bass_utils import run_bass_kernel_spmd

    if "nc" not in _CACHE:
        _CACHE["nc"] = _build()
    nc = _CACHE["nc"]
    in_maps = make_in_maps(inputs)
    _CACHE["in_maps"] = in_maps
    res = run_bass_kernel_spmd(nc, in_maps, core_ids=list(range(NCORES)))
    out = np.empty((B, S), np.float32)
    for core in range(NCORES):
        flat = res.results[core]["out"].reshape(NOUT)
        out[core * BL:(core + 1) * BL, :] = flat.reshape(S, BL).T
    return out


# revision 51
# speedup vs baseline: 1.0172x; 1.0172x over previous
"""DKVMN forward kernel for Trainium2 (8 NeuronCores, batch-parallel).

Per core (8 batches, b = 4*h2 + 2*j + c):
  Phase A: per-tile indirect-DMA gathers, PE transposes -> kT/vT,
    softmax attention w (Exp table load once), gates e/a (Sigmoid/Tanh
    table once).  Compact bf16 images of w and [e|a] are stored to
    DRAM row-major (plain [128, N] copies, ~500ns each).
  Scan (t = 0..199, chunks of CH=20): per chunk 12 contiguous refill
    DMAs (spread over SP/Pool queues) load w9_j [4, CH*128] /
    ea9_j [4, CH*512] bf16 tiles (pad halves zeroed once at startup).
    Per step: 2 bf16 matmuls produce PSUM [A|B] = [w*e | w*a] blocks
    (213ns each), 1 fp32r matmul produces reads psr [8, 512]; DVE does
    T = (A-1)*Mv ; Mv' = B - T (= (1-we)Mv + wa, no ones-row needed);
    one ACT copy drains psr to a bf16 ring, shipped to a DRAM reads
    image every 10 steps (ACT queue).  GPSIMD pulls finished row-tiles
    back as [32,128] partition-aligned loads during the scan.  Steady
    state is DVE-bound at 2x658ns/step; all other engines have slack.
  Phase C: 13 PE transposes + permuted DVE copies build readsT
    [k, (t,b)], fp32r matmuls with f_W halves, Tanh (+bias), p_W
    matvec, Sigmoid, one output DMA of [1, 1600].

  Hardware notes (sim-correct variants that FAIL on real TRN2):
  - multi-column indirect-gather offsets scramble; use [128,1] per tile
  - compute-engine SBUF operands need start partition in {0,32,64,96}
  - fp32r matmul inputs must come from rounding producers (not raw DMA)
  - GPSIMD cannot touch PSUM; f32r memset unsupported
  - partition-dim-split SBUF APs on DMAs lose dependency tracking
"""

import numpy as np

B, S, DK, DV, NQ = 64, 200, 128, 64, 10000
NCORES = 8
BL = B // NCORES          # 8 batches per core
TP = 208                  # t padded to 13*16
NTILE = 13                # row tiles of 128 -> 1664 rows
ROWS = TP * BL            # 1664
NOUT = S * BL             # 1600
CH = 20                   # scan chunk length (steps)
NCH = S // CH
RH = 10                   # reads-ring half length (steps)

_CACHE = {}


def _build():
    import concourse.bacc as bacc
    import concourse.bass as bass
    import concourse.mybir as mybir
    from concourse.tile import TileContext
    from concourse.masks import make_identity

    fp32 = mybir.dt.float32
    f32r = mybir.dt.float32r
    bf16 = mybir.dt.bfloat16
    i32 = mybir.dt.int32
    AL = mybir.AluOpType
    AF = mybir.ActivationFunctionType
    AX = mybir.AxisListType

    nc = bacc.Bacc(None)

    QID = nc.dram_tensor("qid32", [TP, 2 * BL], i32, kind="ExternalInput")
    COR = nc.dram_tensor("cor32", [TP, 2 * BL], i32, kind="ExternalInput")
    KEMB = nc.dram_tensor("key_emb", [NQ, DK], fp32, kind="ExternalInput")
    VEMB = nc.dram_tensor("value_emb", [2 * NQ, DK], fp32, kind="ExternalInput")
    MK = nc.dram_tensor("Mk", [DV, DK], fp32, kind="ExternalInput")
    MV0 = nc.dram_tensor("Mv0", [DV, DK], fp32, kind="ExternalInput")
    FW = nc.dram_tensor("f_W", [2 * DK, DK], fp32, kind="ExternalInput")
    FB = nc.dram_tensor("f_b", [DK], fp32, kind="ExternalInput")
    EW = nc.dram_tensor("e_W", [DK, DK], fp32, kind="ExternalInput")
    EB = nc.dram_tensor("e_b", [DK], fp32, kind="ExternalInput")
    AW = nc.dram_tensor("a_W", [DK, DK], fp32, kind="ExternalInput")
    AB_ = nc.dram_tensor("a_b", [DK], fp32, kind="ExternalInput")
    PW = nc.dram_tensor("p_W", [DK, 1], fp32, kind="ExternalInput")
    PB = nc.dram_tensor("p_b", [1], fp32, kind="ExternalInput")

    # compact bf16 scan-operand images, row-major (8t+b, k); b = 4*h2 + 2*j + c
    WIMG = nc.dram_tensor("w_img", [TP * BL, DV], bf16, kind="Internal")
    EAIMG = nc.dram_tensor("ea_img", [TP * BL, 2 * DK], bf16, kind="Internal")
    # reads image: row = psr row (4*h2+2*j+c), col = (t, g, k)
    RIMG = nc.dram_tensor("r_img", [BL, S * 512], bf16, kind="Internal")
    OUT = nc.dram_tensor("out", [1, NOUT], fp32, kind="ExternalOutput")

    with TileContext(nc) as tc:
        with tc.tile_pool(name="const", bufs=1) as const, \
             tc.tile_pool(name="big", bufs=1) as bigp, \
             tc.tile_pool(name="work", bufs=3) as work, \
             tc.tile_pool(name="small", bufs=4) as small, \
             tc.tile_pool(name="mv", bufs=2) as mvp, \
             tc.tile_pool(name="tt", bufs=2) as ttp, \
             tc.tile_pool(name="stor", bufs=NTILE) as storp:

            psA_cm = tc.tile_pool(name="psA", bufs=2, space="PSUM")
            psA = psA_cm.__enter__()
            psW_cm = tc.tile_pool(name="psW", bufs=2, space="PSUM")
            psW = psW_cm.__enter__()
            gath_cm = tc.tile_pool(name="gath", bufs=1)
            gathp = gath_cm.__enter__()

            # ------- scan tiles + one-time memsets (overlap phase A) -------
            w9 = [[bigp.tile([4, CH * 128], bf16, tag=f"w9_{j}_{i}",
                             name=f"w9_{j}_{i}")
                   for i in range(2)] for j in range(2)]
            ea9 = [[bigp.tile([4, CH * 512], bf16, tag=f"ea9_{j}_{i}",
                              name=f"ea9_{j}_{i}")
                    for i in range(2)] for j in range(2)]
            r8 = [bigp.tile([BL, RH * 512], bf16, tag=f"r8_{i}", name=f"r8_{i}")
                  for i in range(2)]
            rrp = [bigp.tile([128, 128], bf16, tag=f"rrp_{i}", name=f"rrp_{i}")
                   for i in range(NTILE)]
            wsel = bigp.tile([128, ROWS], f32r, tag="wsel")

            # ---------------- constants ----------------
            ident = const.tile([128, 128], fp32, tag="ident")
            make_identity(nc, ident[:])
            ident16 = const.tile([128, 128], bf16, tag="ident16")
            make_identity(nc, ident16[:])

            mk_sb = const.tile([DV, DK], fp32, tag="mk_sb")
            nc.sync.dma_start(mk_sb[:], MK[:])
            mkT_ps = psA.tile([128, 512], fp32, space="PSUM", tag="lps")
            nc.tensor.transpose(mkT_ps[0:DK, 0:DV], mk_sb[:], ident[0:DV, 0:DV])
            mkT = const.tile([DK, DV], f32r, tag="mkT")
            nc.vector.tensor_copy(mkT[:], mkT_ps[0:DK, 0:DV])

            ldw = const.tile([DK, 7 * DK + 1], fp32, tag="ldw")
            nc.sync.dma_start(ldw[:, 0:DK], EW[:])
            nc.sync.dma_start(ldw[:, DK:2 * DK], AW[:])
            nc.sync.dma_start(ldw[:, 2 * DK:3 * DK], FW[0:DK, :])
            nc.sync.dma_start(ldw[:, 3 * DK:4 * DK], FW[DK:2 * DK, :])
            nc.sync.dma_start(ldw[:, 4 * DK:4 * DK + 1], PW[:])
            eaW = const.tile([DK, 2 * DK], f32r, tag="eaW")
            nc.vector.tensor_copy(eaW[:], ldw[:, 0:2 * DK])
            fW1 = const.tile([DK, DK], f32r, tag="fW1")
            nc.vector.tensor_copy(fW1[:], ldw[:, 2 * DK:3 * DK])
            fW2 = const.tile([DK, DK], f32r, tag="fW2")
            nc.vector.tensor_copy(fW2[:], ldw[:, 3 * DK:4 * DK])
            pW = const.tile([DK, 1], f32r, tag="pW")
            nc.vector.tensor_copy(pW[:], ldw[:, 4 * DK:4 * DK + 1])
            nc.sync.dma_start(ldw[0:1, 4 * DK + 1:5 * DK + 1],
                              EB[:].rearrange("(o k) -> o k", o=1))
            nc.sync.dma_start(ldw[0:1, 5 * DK + 1:6 * DK + 1],
                              AB_[:].rearrange("(o k) -> o k", o=1))
            eab_row = const.tile([1, 2 * DK], f32r, tag="eab_row")
            nc.vector.tensor_copy(eab_row[:], ldw[0:1, 4 * DK + 1:6 * DK + 1])
            ones_row = const.tile([1, DK], f32r, tag="ones_row")
            nc.vector.memset(ldw[0:1, 6 * DK + 1:7 * DK + 1], 1.0)
            nc.vector.tensor_copy(ones_row[:], ldw[0:1, 6 * DK + 1:7 * DK + 1])
            fb_col = const.tile([DK, 1], fp32, tag="fb_col")
            nc.sync.dma_start(fb_col[:], FB[:].rearrange("(k o) -> k o", o=1))
            pb_t = const.tile([1, 1], fp32, tag="pb_t")
            nc.sync.dma_start(pb_t[:], PB[:].rearrange("(o k) -> o k", o=1))

            mv0_t = const.tile([128, DK], fp32, tag="mv0_t")
            nc.sync.dma_start(mv0_t[0:DV, :], MV0[:])
            nc.sync.dma_start(mv0_t[DV:128, :], MV0[:])

            # ---------------- indices ----------------
            qidx = const.tile([128, 16], i32, tag="qidx")
            cidx = const.tile([128, 16], i32, tag="cidx")
            vidx = const.tile([128, 16], i32, tag="vidx")
            # row r = 128c + p ; p = 8tl + b ; t = 16c + tl
            qsrc = QID[:].rearrange("(c tl) (b two) -> tl b c two", tl=16, two=2)[:, :, :, 0]
            nc.sync.dma_start(qidx[:, 0:NTILE], qsrc)
            csrc = COR[:].rearrange("(c tl) (b two) -> tl b c two", tl=16, two=2)[:, :, :, 0]
            nc.sync.dma_start(cidx[:, 0:NTILE], csrc)
            nc.vector.scalar_tensor_tensor(
                out=vidx[:, 0:NTILE], in0=cidx[:, 0:NTILE], scalar=NQ,
                in1=qidx[:, 0:NTILE], op0=AL.mult, op1=AL.add)

            # ---------------- batched gathers ----------------
            gk = gathp.tile([128, ROWS], fp32, tag="gk")
            # zero wsel via gk as fp32 staging (f32r memset unsupported);
            # the gathers then overwrite gk (write-after-read tracked)
            nc.vector.memset(gk[:], 0.0)
            nc.vector.tensor_copy(wsel[:], gk[:])
            # w9 zeros are small: DVE, early
            for j in range(2):
                nc.vector.memset(w9[j][0][:], 0.0)
                nc.vector.memset(w9[j][1][:], 0.0)
            nc.vector.memset(rrp[NTILE - 1][:], 0.0)
            gv = gathp.tile([128, ROWS], fp32, tag="gv")

            def gather(c):
                sl = slice(128 * c, 128 * c + 128)
                nc.gpsimd.indirect_dma_start(
                    out=gk[:, sl], out_offset=None, in_=KEMB[:],
                    in_offset=bass.IndirectOffsetOnAxis(ap=qidx[:, c:c + 1], axis=0))
                nc.gpsimd.indirect_dma_start(
                    out=gv[:, sl], out_offset=None, in_=VEMB[:],
                    in_offset=bass.IndirectOffsetOnAxis(ap=vidx[:, c:c + 1], axis=0))

            # Pool order: tiles 0-1 gathers, then buffer-0 ea9 zeros (they
            # gate refill(0)), then the rest, then buffer-1 zeros.
            gather(0)
            gather(1)
            for j in range(2):
                nc.gpsimd.memset(ea9[j][0][:], 0.0)
            for c in range(2, NTILE):
                gather(c)
            for j in range(2):
                nc.gpsimd.memset(ea9[j][1][:], 0.0)

            kT = bigp.tile([DK, ROWS], f32r, tag="kT")
            vT = bigp.tile([DK, ROWS], f32r, tag="vT")

            # ---------------- loop 1: transposes, softmax w ----------------
            for c in range(NTILE):
                sl = slice(128 * c, 128 * c + 128)
                kt_ps = psA.tile([128, 512], fp32, space="PSUM", tag="kt",
                                 bufs=1)
                nc.tensor.transpose(kt_ps[:, 0:128], gk[:, sl], ident[:])
                nc.vector.tensor_copy(kT[:, sl], kt_ps[:, 0:128])
                vt_ps = psA.tile([128, 512], fp32, space="PSUM", tag="vt",
                                 bufs=1)
                nc.tensor.transpose(vt_ps[:, 0:128], gv[:, sl], ident[:])
                nc.vector.tensor_copy(vT[:, sl], vt_ps[:, 0:128])

                lps = psA.tile([128, 512], fp32, space="PSUM", tag="lps")
                nc.tensor.matmul(lps[:, 0:DV], lhsT=kT[:, sl], rhs=mkT[:],
                                 start=True, stop=True)
                negmax = small.tile([128, 1], fp32, tag="nm")
                nc.vector.tensor_reduce(negmax[:], lps[:, 0:DV], AX.X, AL.max,
                                        negate=True)
                exp_sb = work.tile([128, DV], fp32, tag="exp")
                sumexp = small.tile([128, 1], fp32, tag="se")
                nc.scalar.activation(exp_sb[:], lps[:, 0:DV], AF.Exp,
                                     bias=negmax[:, 0:1], accum_out=sumexp[:, 0:1])
                rec = small.tile([128, 1], fp32, tag="rec")
                nc.vector.reciprocal(rec[:], sumexp[:])
                w16 = storp.tile([128, DV], bf16, tag="w16")
                nc.vector.tensor_scalar_mul(w16[:], exp_sb[:], rec[:, 0:1])

                # w image store: plain row-major copy (GPSIMD queue)
                nc.gpsimd.dma_start(WIMG[128 * c:128 * c + 128, :], w16[:])

                # wsel: transpose w16 -> [64, 128], scatter halves by c-parity
                wt_ps = psW.tile([64, 128], bf16, space="PSUM", tag="wps")
                nc.tensor.transpose(wt_ps[:], w16[:], ident16[:])
                wv = wt_ps[:].rearrange("p (t b) -> p t b", b=BL)
                wz = wsel[:, sl].rearrange("p (t b) -> p t b", b=BL)
                nc.vector.tensor_copy(wz[0:DV, :, 0::2], wv[:, :, 0::2])
                nc.vector.tensor_copy(wz[DV:128, :, 1::2], wv[:, :, 1::2])

            gath_cm.__exit__(None, None, None)

            wimg_v = WIMG[:].rearrange("(t h bl) k -> h bl t k", h=2, bl=4)
            eaimg_v = EAIMG[:].rearrange("(t h bl) k -> h bl t k", h=2, bl=4)

            def refill(ch, spread=False, parts="wea"):
                buf = ch % 2
                t0 = ch * CH
                for j in range(2):
                    for h2 in range(2):
                        if "w" in parts:
                            for c2 in range(2):
                                r = 2 * h2 + c2
                                wdst = w9[j][buf][r:r + 1, :].rearrange(
                                    "p (t x) -> p t x", x=128)[
                                    :, :, 64 * c2:64 * c2 + 64]
                                weng = (nc.sync if j == 0 else nc.scalar) \
                                    if spread else nc.gpsimd
                                weng.dma_start(
                                    wdst, wimg_v[h2, 2 * j + c2, t0:t0 + CH, :])
                        if "ea" in parts:
                            eadst = ea9[j][buf][2 * h2:2 * h2 + 2, :].rearrange(
                                "p (t x) -> p t x", x=512)[
                                :, :, 256 * h2:256 * h2 + 256]
                            eng = nc.scalar if (spread and h2 == 1) else nc.sync
                            eng.dma_start(
                                eadst, eaimg_v[h2, 2 * j:2 * j + 2, t0:t0 + CH, :])

            # ---------------- loop 2: gates e|a ----------------
            for c in range(NTILE):
                sl = slice(128 * c, 128 * c + 128)
                eps = psA.tile([128, 512], fp32, space="PSUM", tag="eps")
                nc.tensor.matmul(eps[:, 0:2 * DK], lhsT=vT[:, sl], rhs=eaW[:],
                                 start=True, stop=False)
                nc.tensor.matmul(eps[:, 0:2 * DK], lhsT=ones_row[:], rhs=eab_row[:],
                                 start=False, stop=True)
                ea_sb = storp.tile([128, 2 * DK], bf16, tag="easb")
                nc.scalar.activation(ea_sb[:, 0:DK], eps[:, 0:DK], AF.Sigmoid)
                nc.scalar.activation(ea_sb[:, DK:2 * DK], eps[:, DK:2 * DK], AF.Tanh)
                # ea image store: plain row-major copy (SP queue)
                nc.sync.dma_start(EAIMG[128 * c:128 * c + 128, :], ea_sb[:])
                if c == 1:
                    # chunk-0 refill: only needs image tiles 0-1 (t < 20);
                    # later stores queue behind it harmlessly
                    refill(0, spread=True)

            psW_cm.__exit__(None, None, None)
            psA_cm.__exit__(None, None, None)

            # ---------------- scan init ----------------
            mv_cur = mvp.tile([128, 4 * DK], f32r, tag="mv")
            for g in range(4):
                nc.vector.tensor_copy(mv_cur[:, DK * g:DK * g + DK], mv0_t[:])

            psS_cm = tc.tile_pool(name="psS", bufs=2, space="PSUM")
            psS = psS_cm.__enter__()
            psR_cm = tc.tile_pool(name="psR", bufs=2, space="PSUM")
            psR = psR_cm.__enter__()
            readsT = bigp.tile([DK, NOUT], f32r, tag="readsT")
            rimg_v = RIMG[:].rearrange("r (t g k) -> r t g k", g=4, k=128)

            # ---------------- the scan ----------------
            loaded = 0
            tdone = 0  # all transposes happen in phase C
            for ch in range(NCH):
                buf = ch % 2
                t0 = ch * CH
                for tl in range(CH):
                    t = t0 + tl
                    psab = psS.tile([128, 1024], fp32, space="PSUM", tag="psab")
                    for j in range(2):
                        nc.tensor.matmul(
                            psab[:, 512 * j:512 * j + 512],
                            lhsT=w9[j][buf][0:4, 128 * tl:128 * tl + 128],
                            rhs=ea9[j][buf][0:4, 512 * tl:512 * tl + 512],
                            start=True, stop=True)
                    psr = psR.tile([8, 512], fp32, space="PSUM", tag="psr")
                    nc.tensor.matmul(psr[:], lhsT=wsel[:, 8 * t:8 * t + 8],
                                     rhs=mv_cur[:], start=True, stop=True)

                    psab_v = psab[:].rearrange("p (g x) -> p g x", g=4)
                    tT = ttp.tile([128, 4 * DK], fp32, tag="tt")
                    # T = (A - 1) * Mv
                    nc.vector.scalar_tensor_tensor(
                        out=tT[:].rearrange("p (g x) -> p g x", g=4),
                        in0=psab_v[:, :, 0:DK], scalar=1.0,
                        in1=mv_cur[:].rearrange("p (g x) -> p g x", g=4),
                        op0=AL.subtract, op1=AL.mult)
                    mv_next = mvp.tile([128, 4 * DK], f32r, tag="mv")
                    # Mv' = B - T = (1 - we) Mv + wa
                    nc.vector.tensor_tensor(
                        out=mv_next[:].rearrange("p (g x) -> p g x", g=4),
                        in0=psab_v[:, :, DK:2 * DK],
                        in1=tT[:].rearrange("p (g x) -> p g x", g=4),
                        op=AL.subtract)
                    mv_cur = mv_next

                    # reads drain: one GPSIMD copy into the ring
                    hb = (t // RH) % 2
                    rtl = t % RH
                    nc.scalar.copy(
                        r8[hb][:, 512 * rtl:512 * rtl + 512], psr[:])
                    if rtl == RH - 1:
                        # ship ring half to DRAM (ACT queue; SP carries ea)
                        th = t - RH + 1
                        nc.scalar.dma_start(
                            RIMG[:, 512 * th:512 * (th + RH)], r8[hb][:])

                if ch + 1 < NCH:
                    refill(ch + 1)

                # pull finished row-tiles back (GPSIMD queue)
                cdone = (CH * (ch + 1) - 16) // 16 if ch + 1 < NCH else NTILE - 1
                for ct in range(loaded, cdone + 1):
                    nt = min(16, S - 16 * ct)
                    for h2 in range(2):
                        for j in range(2):
                            rs = 4 * h2 + 2 * j
                            g = 2 * j + h2
                            q = 2 * h2 + j
                            nc.gpsimd.dma_start(
                                rrp[ct][32 * q:32 * q + 2 * nt, :],
                                rimg_v[rs:rs + 2, 16 * ct:16 * ct + nt, g, :])
                loaded = max(loaded, cdone + 1)

            psR_cm.__exit__(None, None, None)
            psS_cm.__exit__(None, None, None)

            # ---------------- phase C ----------------
            psC_cm = tc.tile_pool(name="psC", bufs=3, space="PSUM")
            psC = psC_cm.__enter__()
            psP_cm = tc.tile_pool(name="psP", bufs=2, space="PSUM")
            psP = psP_cm.__enter__()

            for ct in range(tdone, NTILE):
                nt = min(16, S - 16 * ct)
                rt_ps2 = psC.tile([128, 128], bf16, space="PSUM", tag="tps16")
                nc.tensor.transpose(rt_ps2[:], rrp[ct][:], ident16[:])
                tsrc = rt_ps2[:].rearrange("p (q x) -> p q x", q=4)[
                    :, :, 0:2 * nt].rearrange("p q (c t) -> p t q c", c=2)
                tdst = readsT[:, 128 * ct:128 * ct + 8 * nt].rearrange(
                    "p (t q c) -> p t q c", q=4, c=2)
                nc.vector.tensor_copy(tdst, tsrc)

            fT = bigp.tile([DK, NOUT], f32r, tag="fT")
            out_sb = const.tile([1, NOUT], fp32, tag="out_sb")
            for c0 in range(0, NOUT, 512):
                w_ = min(512, NOUT - c0)
                sl = slice(c0, c0 + w_)
                fps = psC.tile([128, 512], fp32, space="PSUM", tag="cps")
                nc.tensor.matmul(fps[:, 0:w_], lhsT=fW1[:], rhs=readsT[:, sl],
                                 start=True, stop=False)
                nc.tensor.matmul(fps[:, 0:w_], lhsT=fW2[:], rhs=kT[:, sl],
                                 start=False, stop=True)
                nc.scalar.activation(fT[:, sl], fps[:, 0:w_], AF.Tanh,
                                     bias=fb_col[:, 0:1])
                pps = psP.tile([1, 512], fp32, space="PSUM", tag="cpr")
                nc.tensor.matmul(pps[0:1, 0:w_], lhsT=pW[:], rhs=fT[:, sl],
                                 start=True, stop=True)
                nc.scalar.activation(out_sb[0:1, sl], pps[0:1, 0:w_], AF.Sigmoid,
                                     bias=pb_t[0:1, 0:1])
            nc.sync.dma_start(OUT[:], out_sb[:])
            psP_cm.__exit__(None, None, None)
            psC_cm.__exit__(None, None, None)

    nc.finalize()
    return nc


def make_in_maps(inputs):
    def prep_idx(a):
        # [BL, S] int -> t-major padded little-endian int32 view [TP, 2*BL]
        a = np.ascontiguousarray(np.asarray(a).astype(np.int64, copy=False).T)  # [S, BL]
        v = a.view(np.int32).reshape(S, 2 * BL)
        out = np.zeros((TP, 2 * BL), np.int32)
        out[:S] = v
        return out

    common = {
        "key_emb": np.ascontiguousarray(inputs["key_emb"], np.float32),
        "value_emb": np.ascontiguousarray(inputs["value_emb"], np.float32),
        "Mk": np.ascontiguousarray(inputs["Mk"], np.float32),
        "Mv0": np.ascontiguousarray(inputs["Mv0"], np.float32),
        "f_W": np.ascontiguousarray(inputs["f_W"], np.float32),
        "f_b": np.ascontiguousarray(inputs["f_b"], np.float32),
        "e_W": np.ascontiguousarray(inputs["e_W"], np.float32),
        "e_b": np.ascontiguousarray(inputs["e_b"], np.float32),
        "a_W": np.ascontiguousarray(inputs["a_W"], np.float32),
        "a_b": np.ascontiguousarray(inputs["a_b"], np.float32),
        "p_W": np.ascontiguousarray(inputs["p_W"], np.float32),
        "p_b": np.ascontiguousarray(inputs["p_b"], np.float32),
    }
    in_maps = []
    for core in range(NCORES):
        bs = slice(core * BL, core * BL + BL)
        m = dict(common)
        m["qid32"] = prep_idx(np.asarray(inputs["question_seq"])[bs])
        m["cor32"] = prep_idx(np.asarray(inputs["correctness_seq"])[bs])
        in_maps.append(m)
    return in_maps


def kernel(**inputs):
    from concourse.bass_utils import run_bass_kernel_spmd

    if "nc" not in _CACHE:
        _CACHE["nc"] = _build()
    nc = _CACHE["nc"]
    in_maps = make_in_maps(inputs)
    _CACHE["in_maps"] = in_maps
    res = run_bass_kernel_spmd(nc, in_maps, core_ids=list(range(NCORES)))
    out = np.empty((B, S), np.float32)
    for core in range(NCORES):
        flat = res.results[core]["out"].reshape(NOUT)
        out[core * BL:(core + 1) * BL, :] = flat.reshape(S, BL).T
    return out


# revision 52
# speedup vs baseline: 1.0385x; 1.0209x over previous
"""DKVMN forward kernel for Trainium2 (8 NeuronCores, batch-parallel).

Per core (8 batches, b = 4*h2 + 2*j + c):
  Phase A: per-tile indirect-DMA gathers, PE transposes -> kT/vT,
    softmax attention w (Exp table load once), gates e/a (Sigmoid/Tanh
    table once).  Compact bf16 images of w and [e|a] are stored to
    DRAM row-major (plain [128, N] copies, ~500ns each).
  Scan (t = 0..199, chunks of CH=20): per chunk 12 contiguous refill
    DMAs (spread over SP/Pool queues) load w9_j [4, CH*128] /
    ea9_j [4, CH*512] bf16 tiles (pad halves zeroed once at startup).
    Per step: 2 bf16 matmuls produce PSUM [A|B] = [w*e | w*a] blocks
    (213ns each), 1 fp32r matmul produces reads psr [8, 512]; DVE does
    T = (A-1)*Mv ; Mv' = B - T (= (1-we)Mv + wa, no ones-row needed);
    one ACT copy drains psr to a bf16 ring, shipped to a DRAM reads
    image every 10 steps (ACT queue).  GPSIMD pulls finished row-tiles
    back as [32,128] partition-aligned loads during the scan.  Steady
    state is DVE-bound at 2x658ns/step; all other engines have slack.
  Phase C: 13 PE transposes + permuted DVE copies build readsT
    [k, (t,b)], fp32r matmuls with f_W halves, Tanh (+bias), p_W
    matvec, Sigmoid, one output DMA of [1, 1600].

  Hardware notes (sim-correct variants that FAIL on real TRN2):
  - multi-column indirect-gather offsets scramble; use [128,1] per tile
  - compute-engine SBUF operands need start partition in {0,32,64,96}
  - fp32r matmul inputs must come from rounding producers (not raw DMA)
  - GPSIMD cannot touch PSUM; f32r memset unsupported
  - partition-dim-split SBUF APs on DMAs lose dependency tracking
"""

import numpy as np

B, S, DK, DV, NQ = 64, 200, 128, 64, 10000
NCORES = 8
BL = B // NCORES          # 8 batches per core
TP = 208                  # t padded to 13*16
NTILE = 13                # row tiles of 128 -> 1664 rows
ROWS = TP * BL            # 1664
NOUT = S * BL             # 1600
CH = 20                   # scan chunk length (steps)
NCH = S // CH
RH = 10                   # reads-ring half length (steps)

_CACHE = {}


def _build():
    import concourse.bacc as bacc
    import concourse.bass as bass
    import concourse.mybir as mybir
    from concourse.tile import TileContext
    from concourse.masks import make_identity

    fp32 = mybir.dt.float32
    f32r = mybir.dt.float32r
    bf16 = mybir.dt.bfloat16
    i32 = mybir.dt.int32
    AL = mybir.AluOpType
    AF = mybir.ActivationFunctionType
    AX = mybir.AxisListType

    nc = bacc.Bacc(None)

    QID = nc.dram_tensor("qid32", [TP, 2 * BL], i32, kind="ExternalInput")
    COR = nc.dram_tensor("cor32", [TP, 2 * BL], i32, kind="ExternalInput")
    KEMB = nc.dram_tensor("key_emb", [NQ, DK], fp32, kind="ExternalInput")
    VEMB = nc.dram_tensor("value_emb", [2 * NQ, DK], fp32, kind="ExternalInput")
    MK = nc.dram_tensor("Mk", [DV, DK], fp32, kind="ExternalInput")
    MV0 = nc.dram_tensor("Mv0", [DV, DK], fp32, kind="ExternalInput")
    FW = nc.dram_tensor("f_W", [2 * DK, DK], fp32, kind="ExternalInput")
    FB = nc.dram_tensor("f_b", [DK], fp32, kind="ExternalInput")
    EW = nc.dram_tensor("e_W", [DK, DK], fp32, kind="ExternalInput")
    EB = nc.dram_tensor("e_b", [DK], fp32, kind="ExternalInput")
    AW = nc.dram_tensor("a_W", [DK, DK], fp32, kind="ExternalInput")
    AB_ = nc.dram_tensor("a_b", [DK], fp32, kind="ExternalInput")
    PW = nc.dram_tensor("p_W", [DK, 1], fp32, kind="ExternalInput")
    PB = nc.dram_tensor("p_b", [1], fp32, kind="ExternalInput")

    # compact bf16 scan-operand images, row-major (8t+b, k); b = 4*h2 + 2*j + c
    WIMG = nc.dram_tensor("w_img", [TP * BL, DV], bf16, kind="Internal")
    EAIMG = nc.dram_tensor("ea_img", [TP * BL, 2 * DK], bf16, kind="Internal")
    # reads image: row = psr row (4*h2+2*j+c), col = (t, g, k)
    RIMG = nc.dram_tensor("r_img", [BL, S * 512], bf16, kind="Internal")
    OUT = nc.dram_tensor("out", [1, NOUT], fp32, kind="ExternalOutput")

    with TileContext(nc) as tc:
        with tc.tile_pool(name="const", bufs=1) as const, \
             tc.tile_pool(name="big", bufs=1) as bigp, \
             tc.tile_pool(name="work", bufs=3) as work, \
             tc.tile_pool(name="small", bufs=4) as small, \
             tc.tile_pool(name="mv", bufs=2) as mvp, \
             tc.tile_pool(name="tt", bufs=2) as ttp, \
             tc.tile_pool(name="stor", bufs=NTILE) as storp:

            psA_cm = tc.tile_pool(name="psA", bufs=2, space="PSUM")
            psA = psA_cm.__enter__()
            psW_cm = tc.tile_pool(name="psW", bufs=2, space="PSUM")
            psW = psW_cm.__enter__()
            gath_cm = tc.tile_pool(name="gath", bufs=1)
            gathp = gath_cm.__enter__()

            # ------- scan tiles + one-time memsets (overlap phase A) -------
            w9 = [[bigp.tile([4, CH * 128], bf16, tag=f"w9_{j}_{i}",
                             name=f"w9_{j}_{i}")
                   for i in range(2)] for j in range(2)]
            ea9 = [[bigp.tile([4, CH * 512], bf16, tag=f"ea9_{j}_{i}",
                              name=f"ea9_{j}_{i}")
                    for i in range(2)] for j in range(2)]
            r8 = [bigp.tile([BL, RH * 512], bf16, tag=f"r8_{i}", name=f"r8_{i}")
                  for i in range(2)]
            rrp = [bigp.tile([128, 128], bf16, tag=f"rrp_{i}", name=f"rrp_{i}")
                   for i in range(NTILE)]
            wsel = bigp.tile([128, ROWS], f32r, tag="wsel")

            # ---------------- constants ----------------
            ident = const.tile([128, 128], fp32, tag="ident")
            make_identity(nc, ident[:])
            ident16 = const.tile([128, 128], bf16, tag="ident16")
            make_identity(nc, ident16[:])

            mk_sb = const.tile([DV, DK], fp32, tag="mk_sb")
            nc.sync.dma_start(mk_sb[:], MK[:])
            mkT_ps = psA.tile([128, 512], fp32, space="PSUM", tag="lps")
            nc.tensor.transpose(mkT_ps[0:DK, 0:DV], mk_sb[:], ident[0:DV, 0:DV])
            mkT = const.tile([DK, DV], f32r, tag="mkT")
            nc.vector.tensor_copy(mkT[:], mkT_ps[0:DK, 0:DV])

            ldw = const.tile([DK, 7 * DK + 1], fp32, tag="ldw")
            nc.sync.dma_start(ldw[:, 0:DK], EW[:])
            nc.sync.dma_start(ldw[:, DK:2 * DK], AW[:])
            nc.sync.dma_start(ldw[:, 2 * DK:3 * DK], FW[0:DK, :])
            nc.sync.dma_start(ldw[:, 3 * DK:4 * DK], FW[DK:2 * DK, :])
            nc.sync.dma_start(ldw[:, 4 * DK:4 * DK + 1], PW[:])
            eaW = const.tile([DK, 2 * DK], f32r, tag="eaW")
            nc.vector.tensor_copy(eaW[:], ldw[:, 0:2 * DK])
            fW1 = const.tile([DK, DK], f32r, tag="fW1")
            nc.vector.tensor_copy(fW1[:], ldw[:, 2 * DK:3 * DK])
            fW2 = const.tile([DK, DK], f32r, tag="fW2")
            nc.vector.tensor_copy(fW2[:], ldw[:, 3 * DK:4 * DK])
            pW = const.tile([DK, 1], f32r, tag="pW")
            nc.vector.tensor_copy(pW[:], ldw[:, 4 * DK:4 * DK + 1])
            nc.sync.dma_start(ldw[0:1, 4 * DK + 1:5 * DK + 1],
                              EB[:].rearrange("(o k) -> o k", o=1))
            nc.sync.dma_start(ldw[0:1, 5 * DK + 1:6 * DK + 1],
                              AB_[:].rearrange("(o k) -> o k", o=1))
            eab_row = const.tile([1, 2 * DK], f32r, tag="eab_row")
            nc.vector.tensor_copy(eab_row[:], ldw[0:1, 4 * DK + 1:6 * DK + 1])
            ones_row = const.tile([1, DK], f32r, tag="ones_row")
            nc.vector.memset(ldw[0:1, 6 * DK + 1:7 * DK + 1], 1.0)
            nc.vector.tensor_copy(ones_row[:], ldw[0:1, 6 * DK + 1:7 * DK + 1])
            fb_col = const.tile([DK, 1], fp32, tag="fb_col")
            nc.sync.dma_start(fb_col[:], FB[:].rearrange("(k o) -> k o", o=1))
            pb_t = const.tile([1, 1], fp32, tag="pb_t")
            nc.sync.dma_start(pb_t[:], PB[:].rearrange("(o k) -> o k", o=1))

            mv0_t = const.tile([128, DK], fp32, tag="mv0_t")
            nc.sync.dma_start(mv0_t[0:DV, :], MV0[:])
            nc.sync.dma_start(mv0_t[DV:128, :], MV0[:])

            # ---------------- indices ----------------
            qidx = const.tile([128, 16], i32, tag="qidx")
            cidx = const.tile([128, 16], i32, tag="cidx")
            vidx = const.tile([128, 16], i32, tag="vidx")
            # row r = 128c + p ; p = 8tl + b ; t = 16c + tl
            qsrc = QID[:].rearrange("(c tl) (b two) -> tl b c two", tl=16, two=2)[:, :, :, 0]
            nc.sync.dma_start(qidx[:, 0:NTILE], qsrc)
            csrc = COR[:].rearrange("(c tl) (b two) -> tl b c two", tl=16, two=2)[:, :, :, 0]
            nc.sync.dma_start(cidx[:, 0:NTILE], csrc)
            nc.vector.scalar_tensor_tensor(
                out=vidx[:, 0:NTILE], in0=cidx[:, 0:NTILE], scalar=NQ,
                in1=qidx[:, 0:NTILE], op0=AL.mult, op1=AL.add)

            # ---------------- batched gathers ----------------
            gk = gathp.tile([128, ROWS], fp32, tag="gk")
            # zero wsel via gk as fp32 staging (f32r memset unsupported);
            # the gathers then overwrite gk (write-after-read tracked)
            nc.vector.memset(gk[:], 0.0)
            nc.vector.tensor_copy(wsel[:], gk[:])
            # w9 zeros are small: DVE, early
            for j in range(2):
                nc.vector.memset(w9[j][0][:], 0.0)
                nc.vector.memset(w9[j][1][:], 0.0)
            nc.vector.memset(rrp[NTILE - 1][:], 0.0)
            gv = gathp.tile([128, ROWS], fp32, tag="gv")

            def gather(c):
                sl = slice(128 * c, 128 * c + 128)
                nc.gpsimd.indirect_dma_start(
                    out=gk[:, sl], out_offset=None, in_=KEMB[:],
                    in_offset=bass.IndirectOffsetOnAxis(ap=qidx[:, c:c + 1], axis=0))
                nc.gpsimd.indirect_dma_start(
                    out=gv[:, sl], out_offset=None, in_=VEMB[:],
                    in_offset=bass.IndirectOffsetOnAxis(ap=vidx[:, c:c + 1], axis=0))

            # Pool order: tiles 0-1 gathers, then buffer-0 ea9 zeros (they
            # gate refill(0)), then the rest, then buffer-1 zeros.
            gather(0)
            gather(1)
            for j in range(2):
                nc.gpsimd.memset(ea9[j][0][:], 0.0)
            for c in range(2, NTILE):
                gather(c)
            for j in range(2):
                nc.gpsimd.memset(ea9[j][1][:], 0.0)

            kT = bigp.tile([DK, ROWS], f32r, tag="kT")
            vT = bigp.tile([DK, ROWS], f32r, tag="vT")

            # ---------------- loop 1: transposes, softmax w ----------------
            for c in range(NTILE):
                sl = slice(128 * c, 128 * c + 128)
                kt_ps = psA.tile([128, 512], fp32, space="PSUM", tag="kt",
                                 bufs=1)
                nc.tensor.transpose(kt_ps[:, 0:128], gk[:, sl], ident[:])
                nc.vector.tensor_copy(kT[:, sl], kt_ps[:, 0:128])
                vt_ps = psA.tile([128, 512], fp32, space="PSUM", tag="vt",
                                 bufs=1)
                nc.tensor.transpose(vt_ps[:, 0:128], gv[:, sl], ident[:])
                nc.vector.tensor_copy(vT[:, sl], vt_ps[:, 0:128])

                lps = psA.tile([128, 512], fp32, space="PSUM", tag="lps")
                nc.tensor.matmul(lps[:, 0:DV], lhsT=kT[:, sl], rhs=mkT[:],
                                 start=True, stop=True)
                negmax = small.tile([128, 1], fp32, tag="nm")
                nc.vector.tensor_reduce(negmax[:], lps[:, 0:DV], AX.X, AL.max,
                                        negate=True)
                exp_sb = work.tile([128, DV], fp32, tag="exp")
                sumexp = small.tile([128, 1], fp32, tag="se")
                nc.scalar.activation(exp_sb[:], lps[:, 0:DV], AF.Exp,
                                     bias=negmax[:, 0:1], accum_out=sumexp[:, 0:1])
                rec = small.tile([128, 1], fp32, tag="rec")
                nc.vector.reciprocal(rec[:], sumexp[:])
                w16 = storp.tile([128, DV], bf16, tag="w16")
                nc.vector.tensor_scalar_mul(w16[:], exp_sb[:], rec[:, 0:1])

                # w image store: plain row-major copy (SP queue -- Pool is
                # clogged with ea9 memsets, and chunk-0 w-refills wait on
                # the last WIMG store)
                nc.sync.dma_start(WIMG[128 * c:128 * c + 128, :], w16[:])

                # wsel: transpose w16 -> [64, 128], scatter halves by c-parity
                wt_ps = psW.tile([64, 128], bf16, space="PSUM", tag="wps")
                nc.tensor.transpose(wt_ps[:], w16[:], ident16[:])
                wv = wt_ps[:].rearrange("p (t b) -> p t b", b=BL)
                wz = wsel[:, sl].rearrange("p (t b) -> p t b", b=BL)
                nc.vector.tensor_copy(wz[0:DV, :, 0::2], wv[:, :, 0::2])
                nc.vector.tensor_copy(wz[DV:128, :, 1::2], wv[:, :, 1::2])

            gath_cm.__exit__(None, None, None)

            wimg_v = WIMG[:].rearrange("(t h bl) k -> h bl t k", h=2, bl=4)
            eaimg_v = EAIMG[:].rearrange("(t h bl) k -> h bl t k", h=2, bl=4)

            def refill(ch, spread=False, parts="wea"):
                buf = ch % 2
                t0 = ch * CH
                for j in range(2):
                    for h2 in range(2):
                        if "w" in parts:
                            for c2 in range(2):
                                r = 2 * h2 + c2
                                wdst = w9[j][buf][r:r + 1, :].rearrange(
                                    "p (t x) -> p t x", x=128)[
                                    :, :, 64 * c2:64 * c2 + 64]
                                weng = (nc.sync if j == 0 else nc.scalar) \
                                    if spread else nc.gpsimd
                                weng.dma_start(
                                    wdst, wimg_v[h2, 2 * j + c2, t0:t0 + CH, :])
                        if "ea" in parts:
                            eadst = ea9[j][buf][2 * h2:2 * h2 + 2, :].rearrange(
                                "p (t x) -> p t x", x=512)[
                                :, :, 256 * h2:256 * h2 + 256]
                            eng = nc.scalar if (spread and h2 == 1) else nc.sync
                            eng.dma_start(
                                eadst, eaimg_v[h2, 2 * j:2 * j + 2, t0:t0 + CH, :])

            # ---------------- loop 2: gates e|a ----------------
            for c in range(NTILE):
                sl = slice(128 * c, 128 * c + 128)
                eps = psA.tile([128, 512], fp32, space="PSUM", tag="eps")
                nc.tensor.matmul(eps[:, 0:2 * DK], lhsT=vT[:, sl], rhs=eaW[:],
                                 start=True, stop=False)
                nc.tensor.matmul(eps[:, 0:2 * DK], lhsT=ones_row[:], rhs=eab_row[:],
                                 start=False, stop=True)
                ea_sb = storp.tile([128, 2 * DK], bf16, tag="easb")
                nc.scalar.activation(ea_sb[:, 0:DK], eps[:, 0:DK], AF.Sigmoid)
                nc.scalar.activation(ea_sb[:, DK:2 * DK], eps[:, DK:2 * DK], AF.Tanh)
                # ea image store: plain row-major copy (SP queue)
                nc.sync.dma_start(EAIMG[128 * c:128 * c + 128, :], ea_sb[:])
                if c == 1:
                    # chunk-0 refill: only needs image tiles 0-1 (t < 20);
                    # later stores queue behind it harmlessly
                    refill(0, spread=True)

            psW_cm.__exit__(None, None, None)
            psA_cm.__exit__(None, None, None)

            # ---------------- scan init ----------------
            mv_cur = mvp.tile([128, 4 * DK], f32r, tag="mv")
            for g in range(4):
                nc.vector.tensor_copy(mv_cur[:, DK * g:DK * g + DK], mv0_t[:])

            psS_cm = tc.tile_pool(name="psS", bufs=2, space="PSUM")
            psS = psS_cm.__enter__()
            psR_cm = tc.tile_pool(name="psR", bufs=2, space="PSUM")
            psR = psR_cm.__enter__()
            readsT = bigp.tile([DK, NOUT], f32r, tag="readsT")
            rimg_v = RIMG[:].rearrange("r (t g k) -> r t g k", g=4, k=128)

            # ---------------- the scan ----------------
            loaded = 0
            tdone = 0  # all transposes happen in phase C
            for ch in range(NCH):
                buf = ch % 2
                t0 = ch * CH
                for tl in range(CH):
                    t = t0 + tl
                    psab = psS.tile([128, 1024], fp32, space="PSUM", tag="psab")
                    for j in range(2):
                        nc.tensor.matmul(
                            psab[:, 512 * j:512 * j + 512],
                            lhsT=w9[j][buf][0:4, 128 * tl:128 * tl + 128],
                            rhs=ea9[j][buf][0:4, 512 * tl:512 * tl + 512],
                            start=True, stop=True)
                    psr = psR.tile([8, 512], fp32, space="PSUM", tag="psr")
                    nc.tensor.matmul(psr[:], lhsT=wsel[:, 8 * t:8 * t + 8],
                                     rhs=mv_cur[:], start=True, stop=True)

                    psab_v = psab[:].rearrange("p (g x) -> p g x", g=4)
                    tT = ttp.tile([128, 4 * DK], fp32, tag="tt")
                    # T = (A - 1) * Mv
                    nc.vector.scalar_tensor_tensor(
                        out=tT[:].rearrange("p (g x) -> p g x", g=4),
                        in0=psab_v[:, :, 0:DK], scalar=1.0,
                        in1=mv_cur[:].rearrange("p (g x) -> p g x", g=4),
                        op0=AL.subtract, op1=AL.mult)
                    mv_next = mvp.tile([128, 4 * DK], f32r, tag="mv")
                    # Mv' = B - T = (1 - we) Mv + wa
                    nc.vector.tensor_tensor(
                        out=mv_next[:].rearrange("p (g x) -> p g x", g=4),
                        in0=psab_v[:, :, DK:2 * DK],
                        in1=tT[:].rearrange("p (g x) -> p g x", g=4),
                        op=AL.subtract)
                    mv_cur = mv_next

                    # reads drain: one GPSIMD copy into the ring
                    hb = (t // RH) % 2
                    rtl = t % RH
                    nc.scalar.copy(
                        r8[hb][:, 512 * rtl:512 * rtl + 512], psr[:])
                    if rtl == RH - 1:
                        # ship ring half to DRAM (ACT queue; SP carries ea)
                        th = t - RH + 1
                        nc.scalar.dma_start(
                            RIMG[:, 512 * th:512 * (th + RH)], r8[hb][:])

                if ch + 1 < NCH:
                    refill(ch + 1)

                # pull finished row-tiles back (GPSIMD queue)
                cdone = (CH * (ch + 1) - 16) // 16 if ch + 1 < NCH else NTILE - 1
                for ct in range(loaded, cdone + 1):
                    nt = min(16, S - 16 * ct)
                    for h2 in range(2):
                        for j in range(2):
                            rs = 4 * h2 + 2 * j
                            g = 2 * j + h2
                            q = 2 * h2 + j
                            nc.gpsimd.dma_start(
                                rrp[ct][32 * q:32 * q + 2 * nt, :],
                                rimg_v[rs:rs + 2, 16 * ct:16 * ct + nt, g, :])
                loaded = max(loaded, cdone + 1)

            psR_cm.__exit__(None, None, None)
            psS_cm.__exit__(None, None, None)

            # ---------------- phase C ----------------
            psC_cm = tc.tile_pool(name="psC", bufs=3, space="PSUM")
            psC = psC_cm.__enter__()
            psP_cm = tc.tile_pool(name="psP", bufs=2, space="PSUM")
            psP = psP_cm.__enter__()

            for ct in range(tdone, NTILE):
                nt = min(16, S - 16 * ct)
                rt_ps2 = psC.tile([128, 128], bf16, space="PSUM", tag="tps16")
                nc.tensor.transpose(rt_ps2[:], rrp[ct][:], ident16[:])
                tsrc = rt_ps2[:].rearrange("p (q x) -> p q x", q=4)[
                    :, :, 0:2 * nt].rearrange("p q (c t) -> p t q c", c=2)
                tdst = readsT[:, 128 * ct:128 * ct + 8 * nt].rearrange(
                    "p (t q c) -> p t q c", q=4, c=2)
                nc.vector.tensor_copy(tdst, tsrc)

            fT = bigp.tile([DK, NOUT], f32r, tag="fT")
            out_sb = const.tile([1, NOUT], fp32, tag="out_sb")
            for c0 in range(0, NOUT, 512):
                w_ = min(512, NOUT - c0)
                sl = slice(c0, c0 + w_)
                fps = psC.tile([128, 512], fp32, space="PSUM", tag="cps")
                nc.tensor.matmul(fps[:, 0:w_], lhsT=fW1[:], rhs=readsT[:, sl],
                                 start=True, stop=False)
                nc.tensor.matmul(fps[:, 0:w_], lhsT=fW2[:], rhs=kT[:, sl],
                                 start=False, stop=True)
                nc.scalar.activation(fT[:, sl], fps[:, 0:w_], AF.Tanh,
                                     bias=fb_col[:, 0:1])
                pps = psP.tile([1, 512], fp32, space="PSUM", tag="cpr")
                nc.tensor.matmul(pps[0:1, 0:w_], lhsT=pW[:], rhs=fT[:, sl],
                                 start=True, stop=True)
                nc.scalar.activation(out_sb[0:1, sl], pps[0:1, 0:w_], AF.Sigmoid,
                                     bias=pb_t[0:1, 0:1])
            nc.sync.dma_start(OUT[:], out_sb[:])
            psP_cm.__exit__(None, None, None)
            psC_cm.__exit__(None, None, None)

    nc.finalize()
    return nc


def make_in_maps(inputs):
    def prep_idx(a):
        # [BL, S] int -> t-major padded little-endian int32 view [TP, 2*BL]
        a = np.ascontiguousarray(np.asarray(a).astype(np.int64, copy=False).T)  # [S, BL]
        v = a.view(np.int32).reshape(S, 2 * BL)
        out = np.zeros((TP, 2 * BL), np.int32)
        out[:S] = v
        return out

    common = {
        "key_emb": np.ascontiguousarray(inputs["key_emb"], np.float32),
        "value_emb": np.ascontiguousarray(inputs["value_emb"], np.float32),
        "Mk": np.ascontiguousarray(inputs["Mk"], np.float32),
        "Mv0": np.ascontiguousarray(inputs["Mv0"], np.float32),
        "f_W": np.ascontiguousarray(inputs["f_W"], np.float32),
        "f_b": np.ascontiguousarray(inputs["f_b"], np.float32),
        "e_W": np.ascontiguousarray(inputs["e_W"], np.float32),
        "e_b": np.ascontiguousarray(inputs["e_b"], np.float32),
        "a_W": np.ascontiguousarray(inputs["a_W"], np.float32),
        "a_b": np.ascontiguousarray(inputs["a_b"], np.float32),
        "p_W": np.ascontiguousarray(inputs["p_W"], np.float32),
        "p_b": np.ascontiguousarray(inputs["p_b"], np.float32),
    }
    in_maps = []
    for core in range(NCORES):
        bs = slice(core * BL, core * BL + BL)
        m = dict(common)
        m["qid32"] = prep_idx(np.asarray(inputs["question_seq"])[bs])
        m["cor32"] = prep_idx(np.asarray(inputs["correctness_seq"])[bs])
        in_maps.append(m)
    return in_maps


def kernel(**inputs):
    from concourse.bass_utils import run_bass_kernel_spmd

    if "nc" not in _CACHE:
        _CACHE["nc"] = _build()
    nc = _CACHE["nc"]
    in_maps = make_in_maps(inputs)
    _CACHE["in_maps"] = in_maps
    res = run_bass_kernel_spmd(nc, in_maps, core_ids=list(range(NCORES)))
    out = np.empty((B, S), np.float32)
    for core in range(NCORES):
        flat = res.results[core]["out"].reshape(NOUT)
        out[core * BL:(core + 1) * BL, :] = flat.reshape(S, BL).T
    return out


# revision 53
# speedup vs baseline: 1.0492x; 1.0102x over previous
"""DKVMN forward kernel for Trainium2 (8 NeuronCores, batch-parallel).

Per core (8 batches, b = 4*h2 + 2*j + c):
  Phase A: per-tile indirect-DMA gathers, PE transposes -> kT/vT,
    softmax attention w (Exp table load once), gates e/a (Sigmoid/Tanh
    table once).  Compact bf16 images of w and [e|a] are stored to
    DRAM row-major (plain [128, N] copies, ~500ns each).
  Scan (t = 0..199, chunks of CH=20): per chunk 12 contiguous refill
    DMAs (spread over SP/Pool queues) load w9_j [4, CH*128] /
    ea9_j [4, CH*512] bf16 tiles (pad halves zeroed once at startup).
    Per step: 2 bf16 matmuls produce PSUM [A|B] = [w*e | w*a] blocks
    (213ns each), 1 fp32r matmul produces reads psr [8, 512]; DVE does
    T = (A-1)*Mv ; Mv' = B - T (= (1-we)Mv + wa, no ones-row needed);
    one ACT copy drains psr to a bf16 ring, shipped to a DRAM reads
    image every 10 steps (ACT queue).  GPSIMD pulls finished row-tiles
    back as [32,128] partition-aligned loads during the scan.  Steady
    state is DVE-bound at 2x658ns/step; all other engines have slack.
  Phase C: 13 PE transposes + permuted DVE copies build readsT
    [k, (t,b)], fp32r matmuls with f_W halves, Tanh (+bias), p_W
    matvec, Sigmoid, one output DMA of [1, 1600].

  Hardware notes (sim-correct variants that FAIL on real TRN2):
  - multi-column indirect-gather offsets scramble; use [128,1] per tile
  - compute-engine SBUF operands need start partition in {0,32,64,96}
  - fp32r matmul inputs must come from rounding producers (not raw DMA)
  - GPSIMD cannot touch PSUM; f32r memset unsupported
  - partition-dim-split SBUF APs on DMAs lose dependency tracking
"""

import numpy as np

B, S, DK, DV, NQ = 64, 200, 128, 64, 10000
NCORES = 8
BL = B // NCORES          # 8 batches per core
TP = 208                  # t padded to 13*16
NTILE = 13                # row tiles of 128 -> 1664 rows
ROWS = TP * BL            # 1664
NOUT = S * BL             # 1600
CH = 20                   # scan chunk length (steps)
NCH = S // CH
RH = 10                   # reads-ring half length (steps)

_CACHE = {}


def _build():
    import concourse.bacc as bacc
    import concourse.bass as bass
    import concourse.mybir as mybir
    from concourse.tile import TileContext
    from concourse.masks import make_identity

    fp32 = mybir.dt.float32
    f32r = mybir.dt.float32r
    bf16 = mybir.dt.bfloat16
    i32 = mybir.dt.int32
    AL = mybir.AluOpType
    AF = mybir.ActivationFunctionType
    AX = mybir.AxisListType

    nc = bacc.Bacc(None)

    QID = nc.dram_tensor("qid32", [TP, 2 * BL], i32, kind="ExternalInput")
    COR = nc.dram_tensor("cor32", [TP, 2 * BL], i32, kind="ExternalInput")
    KEMB = nc.dram_tensor("key_emb", [NQ, DK], fp32, kind="ExternalInput")
    VEMB = nc.dram_tensor("value_emb", [2 * NQ, DK], fp32, kind="ExternalInput")
    MK = nc.dram_tensor("Mk", [DV, DK], fp32, kind="ExternalInput")
    MV0 = nc.dram_tensor("Mv0", [DV, DK], fp32, kind="ExternalInput")
    FW = nc.dram_tensor("f_W", [2 * DK, DK], fp32, kind="ExternalInput")
    FB = nc.dram_tensor("f_b", [DK], fp32, kind="ExternalInput")
    EW = nc.dram_tensor("e_W", [DK, DK], fp32, kind="ExternalInput")
    EB = nc.dram_tensor("e_b", [DK], fp32, kind="ExternalInput")
    AW = nc.dram_tensor("a_W", [DK, DK], fp32, kind="ExternalInput")
    AB_ = nc.dram_tensor("a_b", [DK], fp32, kind="ExternalInput")
    PW = nc.dram_tensor("p_W", [DK, 1], fp32, kind="ExternalInput")
    PB = nc.dram_tensor("p_b", [1], fp32, kind="ExternalInput")

    # compact bf16 scan-operand images, row-major (8t+b, k); b = 4*h2 + 2*j + c
    WIMG = nc.dram_tensor("w_img", [TP * BL, DV], bf16, kind="Internal")
    EAIMG = nc.dram_tensor("ea_img", [TP * BL, 2 * DK], bf16, kind="Internal")
    # reads image: row = psr row (4*h2+2*j+c), col = (t, g, k)
    RIMG = nc.dram_tensor("r_img", [BL, S * 512], bf16, kind="Internal")
    OUT = nc.dram_tensor("out", [1, NOUT], fp32, kind="ExternalOutput")

    with TileContext(nc) as tc:
        with tc.tile_pool(name="const", bufs=1) as const, \
             tc.tile_pool(name="big", bufs=1) as bigp, \
             tc.tile_pool(name="work", bufs=3) as work, \
             tc.tile_pool(name="small", bufs=4) as small, \
             tc.tile_pool(name="mv", bufs=2) as mvp, \
             tc.tile_pool(name="tt", bufs=2) as ttp, \
             tc.tile_pool(name="stor", bufs=NTILE) as storp:

            psA_cm = tc.tile_pool(name="psA", bufs=2, space="PSUM")
            psA = psA_cm.__enter__()
            psW_cm = tc.tile_pool(name="psW", bufs=2, space="PSUM")
            psW = psW_cm.__enter__()
            gath_cm = tc.tile_pool(name="gath", bufs=1)
            gathp = gath_cm.__enter__()

            # ------- scan tiles + one-time memsets (overlap phase A) -------
            w9 = [[bigp.tile([4, CH * 128], bf16, tag=f"w9_{j}_{i}",
                             name=f"w9_{j}_{i}")
                   for i in range(2)] for j in range(2)]
            ea9 = [[bigp.tile([4, CH * 512], bf16, tag=f"ea9_{j}_{i}",
                              name=f"ea9_{j}_{i}")
                    for i in range(2)] for j in range(2)]
            r8 = [bigp.tile([BL, RH * 512], bf16, tag=f"r8_{i}", name=f"r8_{i}")
                  for i in range(2)]
            rrp = [bigp.tile([128, 128], bf16, tag=f"rrp_{i}", name=f"rrp_{i}")
                   for i in range(NTILE)]
            wsel = bigp.tile([128, ROWS], f32r, tag="wsel")

            # ---------------- constants ----------------
            ident = const.tile([128, 128], fp32, tag="ident")
            make_identity(nc, ident[:])
            ident16 = const.tile([128, 128], bf16, tag="ident16")
            make_identity(nc, ident16[:])

            mk_sb = const.tile([DV, DK], fp32, tag="mk_sb")
            nc.sync.dma_start(mk_sb[:], MK[:])
            mkT_ps = psA.tile([128, 512], fp32, space="PSUM", tag="lps")
            nc.tensor.transpose(mkT_ps[0:DK, 0:DV], mk_sb[:], ident[0:DV, 0:DV])
            mkT = const.tile([DK, DV], f32r, tag="mkT")
            nc.vector.tensor_copy(mkT[:], mkT_ps[0:DK, 0:DV])

            ldw = const.tile([DK, 7 * DK + 1], fp32, tag="ldw")
            nc.sync.dma_start(ldw[:, 0:DK], EW[:])
            nc.sync.dma_start(ldw[:, DK:2 * DK], AW[:])
            nc.sync.dma_start(ldw[:, 2 * DK:3 * DK], FW[0:DK, :])
            nc.sync.dma_start(ldw[:, 3 * DK:4 * DK], FW[DK:2 * DK, :])
            nc.sync.dma_start(ldw[:, 4 * DK:4 * DK + 1], PW[:])
            eaW = const.tile([DK, 2 * DK], f32r, tag="eaW")
            nc.vector.tensor_copy(eaW[:], ldw[:, 0:2 * DK])
            fW1 = const.tile([DK, DK], f32r, tag="fW1")
            nc.vector.tensor_copy(fW1[:], ldw[:, 2 * DK:3 * DK])
            fW2 = const.tile([DK, DK], f32r, tag="fW2")
            nc.vector.tensor_copy(fW2[:], ldw[:, 3 * DK:4 * DK])
            pW = const.tile([DK, 1], f32r, tag="pW")
            nc.vector.tensor_copy(pW[:], ldw[:, 4 * DK:4 * DK + 1])
            nc.sync.dma_start(ldw[0:1, 4 * DK + 1:5 * DK + 1],
                              EB[:].rearrange("(o k) -> o k", o=1))
            nc.sync.dma_start(ldw[0:1, 5 * DK + 1:6 * DK + 1],
                              AB_[:].rearrange("(o k) -> o k", o=1))
            eab_row = const.tile([1, 2 * DK], f32r, tag="eab_row")
            nc.vector.tensor_copy(eab_row[:], ldw[0:1, 4 * DK + 1:6 * DK + 1])
            ones_row = const.tile([1, DK], f32r, tag="ones_row")
            nc.vector.memset(ldw[0:1, 6 * DK + 1:7 * DK + 1], 1.0)
            nc.vector.tensor_copy(ones_row[:], ldw[0:1, 6 * DK + 1:7 * DK + 1])
            fb_col = const.tile([DK, 1], fp32, tag="fb_col")
            nc.sync.dma_start(fb_col[:], FB[:].rearrange("(k o) -> k o", o=1))
            pb_t = const.tile([1, 1], fp32, tag="pb_t")
            nc.sync.dma_start(pb_t[:], PB[:].rearrange("(o k) -> o k", o=1))

            mv0_t = const.tile([128, DK], fp32, tag="mv0_t")
            nc.sync.dma_start(mv0_t[0:DV, :], MV0[:])
            nc.sync.dma_start(mv0_t[DV:128, :], MV0[:])

            # ---------------- indices ----------------
            qidx = const.tile([128, 16], i32, tag="qidx")
            cidx = const.tile([128, 16], i32, tag="cidx")
            vidx = const.tile([128, 16], i32, tag="vidx")
            # row r = 128c + p ; p = 8tl + b ; t = 16c + tl
            qsrc = QID[:].rearrange("(c tl) (b two) -> tl b c two", tl=16, two=2)[:, :, :, 0]
            nc.sync.dma_start(qidx[:, 0:NTILE], qsrc)
            csrc = COR[:].rearrange("(c tl) (b two) -> tl b c two", tl=16, two=2)[:, :, :, 0]
            nc.sync.dma_start(cidx[:, 0:NTILE], csrc)
            nc.vector.scalar_tensor_tensor(
                out=vidx[:, 0:NTILE], in0=cidx[:, 0:NTILE], scalar=NQ,
                in1=qidx[:, 0:NTILE], op0=AL.mult, op1=AL.add)

            # ---------------- batched gathers ----------------
            gk = gathp.tile([128, ROWS], fp32, tag="gk")
            # zero wsel via gk as fp32 staging (f32r memset unsupported);
            # the gathers then overwrite gk (write-after-read tracked)
            nc.vector.memset(gk[:], 0.0)
            nc.vector.tensor_copy(wsel[:], gk[:])
            # w9 zeros are small: DVE, early
            for j in range(2):
                nc.vector.memset(w9[j][0][:].bitcast(fp32), 0.0)
                nc.vector.memset(w9[j][1][:].bitcast(fp32), 0.0)
            nc.vector.memset(rrp[NTILE - 1][:], 0.0)
            gv = gathp.tile([128, ROWS], fp32, tag="gv")

            def gather(c):
                sl = slice(128 * c, 128 * c + 128)
                nc.gpsimd.indirect_dma_start(
                    out=gk[:, sl], out_offset=None, in_=KEMB[:],
                    in_offset=bass.IndirectOffsetOnAxis(ap=qidx[:, c:c + 1], axis=0))
                nc.gpsimd.indirect_dma_start(
                    out=gv[:, sl], out_offset=None, in_=VEMB[:],
                    in_offset=bass.IndirectOffsetOnAxis(ap=vidx[:, c:c + 1], axis=0))

            # Pool order: tiles 0-1 gathers, then buffer-0 ea9 zeros (they
            # gate refill(0)), then the rest, then buffer-1 zeros.
            gather(0)
            gather(1)
            for j in range(2):
                nc.gpsimd.memset(ea9[j][0][:].bitcast(fp32), 0.0)
            for c in range(2, NTILE):
                gather(c)
            for j in range(2):
                nc.gpsimd.memset(ea9[j][1][:].bitcast(fp32), 0.0)

            kT = bigp.tile([DK, ROWS], f32r, tag="kT")
            vT = bigp.tile([DK, ROWS], f32r, tag="vT")

            # ---------------- loop 1: transposes, softmax w ----------------
            for c in range(NTILE):
                sl = slice(128 * c, 128 * c + 128)
                kt_ps = psA.tile([128, 512], fp32, space="PSUM", tag="kt",
                                 bufs=1)
                nc.tensor.transpose(kt_ps[:, 0:128], gk[:, sl], ident[:])
                nc.vector.tensor_copy(kT[:, sl], kt_ps[:, 0:128])
                vt_ps = psA.tile([128, 512], fp32, space="PSUM", tag="vt",
                                 bufs=1)
                nc.tensor.transpose(vt_ps[:, 0:128], gv[:, sl], ident[:])
                nc.vector.tensor_copy(vT[:, sl], vt_ps[:, 0:128])

                lps = psA.tile([128, 512], fp32, space="PSUM", tag="lps")
                nc.tensor.matmul(lps[:, 0:DV], lhsT=kT[:, sl], rhs=mkT[:],
                                 start=True, stop=True)
                negmax = small.tile([128, 1], fp32, tag="nm")
                nc.vector.tensor_reduce(negmax[:], lps[:, 0:DV], AX.X, AL.max,
                                        negate=True)
                exp_sb = work.tile([128, DV], fp32, tag="exp")
                sumexp = small.tile([128, 1], fp32, tag="se")
                nc.scalar.activation(exp_sb[:], lps[:, 0:DV], AF.Exp,
                                     bias=negmax[:, 0:1], accum_out=sumexp[:, 0:1])
                rec = small.tile([128, 1], fp32, tag="rec")
                nc.vector.reciprocal(rec[:], sumexp[:])
                w16 = storp.tile([128, DV], bf16, tag="w16")
                nc.vector.tensor_scalar_mul(w16[:], exp_sb[:], rec[:, 0:1])

                # w image store: plain row-major copy (SP queue -- Pool is
                # clogged with ea9 memsets, and chunk-0 w-refills wait on
                # the last WIMG store)
                nc.sync.dma_start(WIMG[128 * c:128 * c + 128, :], w16[:])

                # wsel: transpose w16 -> [64, 128], scatter halves by c-parity
                wt_ps = psW.tile([64, 128], bf16, space="PSUM", tag="wps")
                nc.tensor.transpose(wt_ps[:], w16[:], ident16[:])
                wv = wt_ps[:].rearrange("p (t b) -> p t b", b=BL)
                wz = wsel[:, sl].rearrange("p (t b) -> p t b", b=BL)
                nc.vector.tensor_copy(wz[0:DV, :, 0::2], wv[:, :, 0::2])
                nc.vector.tensor_copy(wz[DV:128, :, 1::2], wv[:, :, 1::2])

            gath_cm.__exit__(None, None, None)

            wimg_v = WIMG[:].rearrange("(t h bl) k -> h bl t k", h=2, bl=4)
            eaimg_v = EAIMG[:].rearrange("(t h bl) k -> h bl t k", h=2, bl=4)

            def refill(ch, spread=False, parts="wea"):
                buf = ch % 2
                t0 = ch * CH
                for j in range(2):
                    for h2 in range(2):
                        if "w" in parts:
                            for c2 in range(2):
                                r = 2 * h2 + c2
                                wdst = w9[j][buf][r:r + 1, :].rearrange(
                                    "p (t x) -> p t x", x=128)[
                                    :, :, 64 * c2:64 * c2 + 64]
                                weng = (nc.sync if j == 0 else nc.scalar) \
                                    if spread else nc.gpsimd
                                weng.dma_start(
                                    wdst, wimg_v[h2, 2 * j + c2, t0:t0 + CH, :])
                        if "ea" in parts:
                            eadst = ea9[j][buf][2 * h2:2 * h2 + 2, :].rearrange(
                                "p (t x) -> p t x", x=512)[
                                :, :, 256 * h2:256 * h2 + 256]
                            eng = nc.scalar if (spread and h2 == 1) else nc.sync
                            eng.dma_start(
                                eadst, eaimg_v[h2, 2 * j:2 * j + 2, t0:t0 + CH, :])

            # ---------------- loop 2: gates e|a ----------------
            for c in range(NTILE):
                sl = slice(128 * c, 128 * c + 128)
                eps = psA.tile([128, 512], fp32, space="PSUM", tag="eps")
                nc.tensor.matmul(eps[:, 0:2 * DK], lhsT=vT[:, sl], rhs=eaW[:],
                                 start=True, stop=False)
                nc.tensor.matmul(eps[:, 0:2 * DK], lhsT=ones_row[:], rhs=eab_row[:],
                                 start=False, stop=True)
                ea_sb = storp.tile([128, 2 * DK], bf16, tag="easb")
                nc.scalar.activation(ea_sb[:, 0:DK], eps[:, 0:DK], AF.Sigmoid)
                nc.scalar.activation(ea_sb[:, DK:2 * DK], eps[:, DK:2 * DK], AF.Tanh)
                # ea image store: plain row-major copy (SP queue)
                nc.sync.dma_start(EAIMG[128 * c:128 * c + 128, :], ea_sb[:])
                if c == 1:
                    # chunk-0 refill: only needs image tiles 0-1 (t < 20);
                    # later stores queue behind it harmlessly
                    refill(0, spread=True)

            psW_cm.__exit__(None, None, None)
            psA_cm.__exit__(None, None, None)

            # ---------------- scan init ----------------
            mv_cur = mvp.tile([128, 4 * DK], f32r, tag="mv")
            for g in range(4):
                nc.vector.tensor_copy(mv_cur[:, DK * g:DK * g + DK], mv0_t[:])

            psS_cm = tc.tile_pool(name="psS", bufs=2, space="PSUM")
            psS = psS_cm.__enter__()
            psR_cm = tc.tile_pool(name="psR", bufs=2, space="PSUM")
            psR = psR_cm.__enter__()
            readsT = bigp.tile([DK, NOUT], f32r, tag="readsT")
            rimg_v = RIMG[:].rearrange("r (t g k) -> r t g k", g=4, k=128)

            # ---------------- the scan ----------------
            loaded = 0
            tdone = 0  # all transposes happen in phase C
            for ch in range(NCH):
                buf = ch % 2
                t0 = ch * CH
                for tl in range(CH):
                    t = t0 + tl
                    psab = psS.tile([128, 1024], fp32, space="PSUM", tag="psab")
                    for j in range(2):
                        nc.tensor.matmul(
                            psab[:, 512 * j:512 * j + 512],
                            lhsT=w9[j][buf][0:4, 128 * tl:128 * tl + 128],
                            rhs=ea9[j][buf][0:4, 512 * tl:512 * tl + 512],
                            start=True, stop=True)
                    psr = psR.tile([8, 512], fp32, space="PSUM", tag="psr")
                    nc.tensor.matmul(psr[:], lhsT=wsel[:, 8 * t:8 * t + 8],
                                     rhs=mv_cur[:], start=True, stop=True)

                    psab_v = psab[:].rearrange("p (g x) -> p g x", g=4)
                    tT = ttp.tile([128, 4 * DK], fp32, tag="tt")
                    # T = (A - 1) * Mv
                    nc.vector.scalar_tensor_tensor(
                        out=tT[:].rearrange("p (g x) -> p g x", g=4),
                        in0=psab_v[:, :, 0:DK], scalar=1.0,
                        in1=mv_cur[:].rearrange("p (g x) -> p g x", g=4),
                        op0=AL.subtract, op1=AL.mult)
                    mv_next = mvp.tile([128, 4 * DK], f32r, tag="mv")
                    # Mv' = B - T = (1 - we) Mv + wa
                    nc.vector.tensor_tensor(
                        out=mv_next[:].rearrange("p (g x) -> p g x", g=4),
                        in0=psab_v[:, :, DK:2 * DK],
                        in1=tT[:].rearrange("p (g x) -> p g x", g=4),
                        op=AL.subtract)
                    mv_cur = mv_next

                    # reads drain: one GPSIMD copy into the ring
                    hb = (t // RH) % 2
                    rtl = t % RH
                    nc.scalar.copy(
                        r8[hb][:, 512 * rtl:512 * rtl + 512], psr[:])
                    if rtl == RH - 1:
                        # ship ring half to DRAM (ACT queue; SP carries ea)
                        th = t - RH + 1
                        nc.scalar.dma_start(
                            RIMG[:, 512 * th:512 * (th + RH)], r8[hb][:])

                if ch + 1 < NCH:
                    refill(ch + 1)

                # pull finished row-tiles back (GPSIMD queue)
                cdone = (CH * (ch + 1) - 16) // 16 if ch + 1 < NCH else NTILE - 1
                for ct in range(loaded, cdone + 1):
                    nt = min(16, S - 16 * ct)
                    for h2 in range(2):
                        for j in range(2):
                            rs = 4 * h2 + 2 * j
                            g = 2 * j + h2
                            q = 2 * h2 + j
                            nc.gpsimd.dma_start(
                                rrp[ct][32 * q:32 * q + 2 * nt, :],
                                rimg_v[rs:rs + 2, 16 * ct:16 * ct + nt, g, :])
                loaded = max(loaded, cdone + 1)

            psR_cm.__exit__(None, None, None)
            psS_cm.__exit__(None, None, None)

            # ---------------- phase C ----------------
            psC_cm = tc.tile_pool(name="psC", bufs=3, space="PSUM")
            psC = psC_cm.__enter__()
            psP_cm = tc.tile_pool(name="psP", bufs=2, space="PSUM")
            psP = psP_cm.__enter__()

            for ct in range(tdone, NTILE):
                nt = min(16, S - 16 * ct)
                rt_ps2 = psC.tile([128, 128], bf16, space="PSUM", tag="tps16")
                nc.tensor.transpose(rt_ps2[:], rrp[ct][:], ident16[:])
                tsrc = rt_ps2[:].rearrange("p (q x) -> p q x", q=4)[
                    :, :, 0:2 * nt].rearrange("p q (c t) -> p t q c", c=2)
                tdst = readsT[:, 128 * ct:128 * ct + 8 * nt].rearrange(
                    "p (t q c) -> p t q c", q=4, c=2)
                nc.vector.tensor_copy(tdst, tsrc)

            fT = bigp.tile([DK, NOUT], f32r, tag="fT")
            out_sb = const.tile([1, NOUT], fp32, tag="out_sb")
            for c0 in range(0, NOUT, 512):
                w_ = min(512, NOUT - c0)
                sl = slice(c0, c0 + w_)
                fps = psC.tile([128, 512], fp32, space="PSUM", tag="cps")
                nc.tensor.matmul(fps[:, 0:w_], lhsT=fW1[:], rhs=readsT[:, sl],
                                 start=True, stop=False)
                nc.tensor.matmul(fps[:, 0:w_], lhsT=fW2[:], rhs=kT[:, sl],
                                 start=False, stop=True)
                nc.scalar.activation(fT[:, sl], fps[:, 0:w_], AF.Tanh,
                                     bias=fb_col[:, 0:1])
                pps = psP.tile([1, 512], fp32, space="PSUM", tag="cpr")
                nc.tensor.matmul(pps[0:1, 0:w_], lhsT=pW[:], rhs=fT[:, sl],
                                 start=True, stop=True)
                nc.scalar.activation(out_sb[0:1, sl], pps[0:1, 0:w_], AF.Sigmoid,
                                     bias=pb_t[0:1, 0:1])
            nc.sync.dma_start(OUT[:], out_sb[:])
            psP_cm.__exit__(None, None, None)
            psC_cm.__exit__(None, None, None)

    nc.finalize()
    return nc


def make_in_maps(inputs):
    def prep_idx(a):
        # [BL, S] int -> t-major padded little-endian int32 view [TP, 2*BL]
        a = np.ascontiguousarray(np.asarray(a).astype(np.int64, copy=False).T)  # [S, BL]
        v = a.view(np.int32).reshape(S, 2 * BL)
        out = np.zeros((TP, 2 * BL), np.int32)
        out[:S] = v
        return out

    common = {
        "key_emb": np.ascontiguousarray(inputs["key_emb"], np.float32),
        "value_emb": np.ascontiguousarray(inputs["value_emb"], np.float32),
        "Mk": np.ascontiguousarray(inputs["Mk"], np.float32),
        "Mv0": np.ascontiguousarray(inputs["Mv0"], np.float32),
        "f_W": np.ascontiguousarray(inputs["f_W"], np.float32),
        "f_b": np.ascontiguousarray(inputs["f_b"], np.float32),
        "e_W": np.ascontiguousarray(inputs["e_W"], np.float32),
        "e_b": np.ascontiguousarray(inputs["e_b"], np.float32),
        "a_W": np.ascontiguousarray(inputs["a_W"], np.float32),
        "a_b": np.ascontiguousarray(inputs["a_b"], np.float32),
        "p_W": np.ascontiguousarray(inputs["p_W"], np.float32),
        "p_b": np.ascontiguousarray(inputs["p_b"], np.float32),
    }
    in_maps = []
    for core in range(NCORES):
        bs = slice(core * BL, core * BL + BL)
        m = dict(common)
        m["qid32"] = prep_idx(np.asarray(inputs["question_seq"])[bs])
        m["cor32"] = prep_idx(np.asarray(inputs["correctness_seq"])[bs])
        in_maps.append(m)
    return in_maps


def kernel(**inputs):
    from concourse.bass_utils import run_bass_kernel_spmd

    if "nc" not in _CACHE:
        _CACHE["nc"] = _build()
    nc = _CACHE["nc"]
    in_maps = make_in_maps(inputs)
    _CACHE["in_maps"] = in_maps
    res = run_bass_kernel_spmd(nc, in_maps, core_ids=list(range(NCORES)))
    out = np.empty((B, S), np.float32)
    for core in range(NCORES):
        flat = res.results[core]["out"].reshape(NOUT)
        out[core * BL:(core + 1) * BL, :] = flat.reshape(S, BL).T
    return out
